# revision 25
# baseline (speedup 1.0000x reference)
"""CBAM kernel for Trainium2, 8-core data-parallel (4 batches per core).

Layout trick: per core the shard is [12544, 256] (4 batches x 3136 spatial x 256ch).
Split into 2 batch-PAIRS of [6272, 256]. Within a pair, flat row r = 49*p + n
(p in [0,128), n in [0,49)) puts batch = p//64 exactly on a 64-partition group
(3136 = 64*49), giving fully contiguous per-partition DMA (50KB runs) and
letting every compute op span all 128 partitions.

v3: bf16 data plane. gpsimd-issued CASTING DMAs convert f32->bf16 on the
way in and bf16->f32 on the way out, so every bulk DVE op runs on 2-byte
data (2x_1p mode) and SBUF traffic halves:
  - in: x lands as bf16 (the only copy held on chip; xg overwrites it).
  - chmax chain + smax reduce + phase4 scalar-mul: all-bf16 DVE ops (2x).
  - phase2: per-block scalar_tensor_tensor fuses xg=x*cg with the savg
    sum accumulation (accum register is fp32, exempt from the 2x rule).
  - chsum: PE bf16 matmuls (1 cyc/row) against a bf16 0/1 mask, two
    256-col blocks per matmul (512 moving cols), fp32 PSUM accumulate.
  - savg stays fp32 (sum accuracy); smax travels bf16 through its DRAM
    shuffle and is cast to f32 on the read-back; conv/MLP stay fp32.
  - savg is a raw channel SUM; the 1/C of the mean is folded into the
    ic=0 rows of the conv band matrices on the host.
  - phase4 pair0 on ACT (overlaps DVE's phase2_1), pair1 on DVE.
  - DMA engine budget: HWDGE (sync/scalar queues) only carries the tiny
    const loads; all bulk traffic is SWDGE via gpsimd (casting).
Rounding cost: |rel err| ~ 4e-3 (bf16 x, bf16 xg, bf16 out) against the
f32 reference -- comfortably inside the 2e-2 gate.
"""

import numpy as np
from contextlib import ExitStack

import concourse.bass as bass
import concourse.tile as tile
from concourse import mybir
from concourse._compat import with_exitstack
from concourse.tile import add_dep_helper

F32 = mybir.dt.float32
BF16 = mybir.dt.bfloat16

C = 256
HID = 16
NPAIR = 2          # batch pairs per core
NBLK = 49          # 256-ch blocks per pair free dim (3136 = 64*49)
CHUNK = 7          # blocks per DMA chunk
NCHUNK = NBLK // CHUNK
ROWS_PAIR = 128 * NBLK   # 6272
ROWS_CORE = NPAIR * ROWS_PAIR  # 12544
H = W = 56
SP = H * W  # 3136
N_CORES = 8

MU = mybir.AluOpType
AF = mybir.ActivationFunctionType


def _ap(handle_ap, offset_elems, dims):
    """Raw AP over a DRAM tensor: dims = [[step, count], ...] in elements."""
    base = handle_ap[tuple([slice(None)] * len(handle_ap.shape))]
    return bass.AP(tensor=base.tensor, offset=base.offset + offset_elems, ap=dims)


@with_exitstack
def _emit(ctx: ExitStack, tc: tile.TileContext):
    nc = tc.nc

    x_d = nc.dram_tensor("x", [ROWS_CORE, C], F32, kind="ExternalInput")
    w1h_d = nc.dram_tensor("w1h", [128, 2, HID], F32, kind="ExternalInput")
    w1sh_d = nc.dram_tensor("w1sh", [128, 2, HID], F32, kind="ExternalInput")
    w2h_d = nc.dram_tensor("w2h", [HID, 2, 128], F32, kind="ExternalInput")
    b1c_d = nc.dram_tensor("b1c", [HID, 1], F32, kind="ExternalInput")
    b2t_d = nc.dram_tensor("b2t", [128, 2], F32, kind="ExternalInput")
    bands_d = nc.dram_tensor("bands", [H, 14, H], F32, kind="ExternalInput")
    ident_d = nc.dram_tensor("ident", [128, 128], F32, kind="ExternalInput")
    ident16_d = nc.dram_tensor("ident16", [128, 128], BF16, kind="ExternalInput")
    mask2_d = nc.dram_tensor("mask2", [128, 2], BF16, kind="ExternalInput")
    mask2t_d = nc.dram_tensor("mask2t", [2, 128], F32, kind="ExternalInput")
    convb_d = nc.dram_tensor("convb", [H, 1], F32, kind="ExternalInput")
    out_d = nc.dram_tensor("out", [ROWS_CORE, C], F32, kind="ExternalOutput")

    # DRAM scratch for the conv-input / spatial-gate reshuffles
    savg_d = nc.dram_tensor("savg_s", [NPAIR, ROWS_PAIR], F32)
    smax_d = nc.dram_tensor("smax_s", [NPAIR, ROWS_PAIR], BF16)
    sg_d = nc.dram_tensor("sg_s", [NPAIR, ROWS_PAIR], F32)

    xv = x_d[:, :].rearrange("(q p n) c -> q p n c", q=NPAIR, p=128)
    ov = out_d[:, :].rearrange("(q p n) c -> q p n c", q=NPAIR, p=128)

    constp = ctx.enter_context(tc.tile_pool(name="const", bufs=1))
    bigp = ctx.enter_context(tc.tile_pool(name="big", bufs=1))
    workp = ctx.enter_context(tc.tile_pool(name="work", bufs=1))
    psp1 = ctx.enter_context(tc.tile_pool(name="ps1", bufs=1, space="PSUM"))
    psp2 = ctx.enter_context(tc.tile_pool(name="ps2", bufs=2, space="PSUM"))

    # ---- casting in-DMAs (gpsimd SWDGE): f32 DRAM -> bf16 SBUF ----
    X = {}
    OS = {}
    for q in range(NPAIR):
        X[q] = bigp.tile([128, NBLK, C], BF16, tag=f"x{q}", name=f"x{q}")
        OS[q] = bigp.tile([128, NBLK, C], F32, tag=f"os{q}", name=f"os{q}")
    for q in range(NPAIR):
        for k in range(NCHUNK):
            nc.gpsimd.dma_start(
                X[q][:, k * CHUNK : (k + 1) * CHUNK, :],
                xv[q, :, k * CHUNK : (k + 1) * CHUNK, :],
            )

    # ---- constants via the sync/scalar HWDGE queues (free early) ----
    def const_load(name, shape, dram, dtype=F32, eng=None):
        t = constp.tile(shape, dtype, tag=name, name=name)
        (eng or nc.scalar).dma_start(t[tuple([slice(None)] * len(shape))], dram)
        return t

    # chsum / chmax gates first (PE + transposes touch these earliest)
    mask2 = const_load("mask2", [128, 2], mask2_d[:, :], dtype=BF16,
                       eng=nc.sync)
    ident16 = const_load("ident16", [128, 128], ident16_d[:, :], dtype=BF16,
                         eng=nc.sync)
    ident = const_load("ident", [128, 128], ident_d[:, :], eng=nc.sync)
    w1h = const_load("w1h", [128, 2, HID], w1h_d[:, :, :])
    w1sh = const_load("w1sh", [128, 2, HID], w1sh_d[:, :, :])
    w2h = const_load("w2h", [HID, 2, 128], w2h_d[:, :, :])
    b1c = const_load("b1c", [HID, 1], b1c_d[:, :])
    b2t = const_load("b2t", [128, 2], b2t_d[:, :])
    mask2t = const_load("mask2t", [2, 128], mask2t_d[:, :])
    bands = const_load("bands", [H, 14, H], bands_d[:, :, :])
    convb = const_load("convb", [H, 1], convb_d[:, :])

    # DVE funnel copies: every fp32 matmul operand must reach PE with deps on
    # at most one engine (fused-LDWEIGHTS fp32 matmuls tolerate 1 sync wait).
    def funnel(name, src, shape):
        t = constp.tile(shape, F32, tag=name)
        nc.vector.tensor_copy(t[tuple([slice(None)] * len(shape))],
                              src[tuple([slice(None)] * len(shape))])
        return t

    identb = funnel("identb", ident, [128, 128])
    w1hb = funnel("w1hb", w1h, [128, 2, HID])
    w1shb = funnel("w1shb", w1sh, [128, 2, HID])
    w2hb = funnel("w2hb", w2h, [HID, 2, 128])
    bandsb = funnel("bandsb", bands, [H, 14, H])
    mask2tb = funnel("mask2tb", mask2t, [2, 128])

    prev = {}
    aw = {q: workp.tile([128, CHUNK, C], BF16, tag=f"aw{q}", name=f"aw{q}")
          for q in range(NPAIR)}
    chsum = {}

    def phase1_chunk(q, k):
        """chmax chain step on DVE (bf16, 2x) + chsum on PE (bf16, two
        blocks per matmul = 512 moving cols, fp32 PSUM accumulate)."""
        if k == 0:
            chsum[q] = psp2.tile([2, 2, C], F32, tag="chsum", name="chsum")
        blk = X[q][:, k * CHUNK : (k + 1) * CHUNK, :]
        if k == 0:
            nc.vector.tensor_copy(aw[q][:], blk)
        else:
            nc.vector.tensor_max(aw[q][:], aw[q][:], blk)
        n0 = k * CHUNK
        for j in range(3):
            mm = nc.tensor.matmul(
                chsum[q][:], lhsT=mask2[:],
                rhs=X[q][:, n0 + 2 * j : n0 + 2 * j + 2, :],
                start=(k == 0 and j == 0), stop=False,
                skip_group_check=True,
            )
            if k == 0 and j == 0:
                if "last_chsum" in prev:
                    add_dep_helper(mm.ins, prev["last_chsum"].ins, sync=False,
                                   reason="pair order on PE")
        # odd 7th block accumulates into the first half
        mm = nc.tensor.matmul(
            chsum[q][:, 0, :], lhsT=mask2[:], rhs=X[q][:, n0 + 6, :],
            start=False, stop=(k == NCHUNK - 1),
            skip_group_check=True,
        )
        if k == NCHUNK - 1:
            prev["last_chsum"] = mm

    def phase1_folds(q):
        a = aw[q]
        nc.vector.tensor_max(a[:, 0:3, :], a[:, 0:3, :], a[:, 3:6, :])
        nc.vector.tensor_max(a[:, 0, :], a[:, 0, :], a[:, 1, :])
        nc.vector.tensor_max(a[:, 0, :], a[:, 0, :], a[:, 2, :])
        nc.vector.tensor_max(a[:, 0, :], a[:, 0, :], a[:, 6, :])
        return a[:, 0, :], chsum[q]

    def mlp(q, acc, chsum_ps):
        # statsT[c_in_half, half, stat(avg=0,max=1), b]
        statsT = workp.tile([128, 2, 2, 2], F32, tag=f"stats{q}")
        # fold the two psum halves -> channel sums [2, C]
        sum_sb = workp.tile([2, C], F32, tag=f"sum{q}")
        nc.vector.tensor_copy(sum_sb[:], chsum_ps[:, 0, :])
        nc.vector.tensor_add(sum_sb[:], sum_sb[:], chsum_ps[:, 1, :])
        mlp_ps = psp1.tile([128, 16], F32, tag="mlp")
        for h2 in range(2):
            tp = psp1.tile([128, 128], BF16, tag="tp")
            nc.tensor.transpose(tp[:], acc[:, h2 * 128 : (h2 + 1) * 128],
                                ident16[:])
            nc.vector.tensor_reduce(
                out=statsT[:, h2, 1, :],
                in_=tp[:].rearrange("c (b p) -> c b p", b=2),
                axis=mybir.AxisListType.X, op=MU.max,
            )
            nc.tensor.transpose(
                mlp_ps[:, 2 * h2 : 2 * h2 + 2],
                sum_sb[:, h2 * 128 : (h2 + 1) * 128],
                identb[0:2, 0:2],
            )
            nc.vector.tensor_copy(
                statsT[:, h2, 0, :], mlp_ps[:, 2 * h2 : 2 * h2 + 2]
            )

        for stat in range(2):
            w1x = w1shb if stat == 0 else w1hb
            for h2 in range(2):
                nc.tensor.matmul(
                    mlp_ps[0:HID, 4 + 2 * stat : 6 + 2 * stat],
                    lhsT=w1x[:, h2, :], rhs=statsT[:, h2, stat, :],
                    start=(h2 == 0), stop=(h2 == 1),
                )
        h_sb = workp.tile([HID, 2, 2], F32, tag=f"hsb{q}")
        # h = max(h_ps + b1, 0)  (relu on DVE to keep ACT tables stable)
        nc.vector.tensor_scalar(
            out=h_sb[:], in0=mlp_ps[0:HID, 4:8].rearrange("p (s b) -> p s b", s=2),
            scalar1=b1c[:], scalar2=0.0,
            op0=MU.add, op1=MU.max,
        )
        sigT = workp.tile([128, 2, 4], F32, tag=f"sig{q}")
        for h2 in range(2):
            cgp = mlp_ps[:, 8 + 4 * h2 : 12 + 4 * h2]
            nc.tensor.matmul(
                cgp, lhsT=w2hb[:, h2, :], rhs=h_sb[:, :, :],
                start=True, stop=True,
            )
            nc.scalar.activation(
                out=sigT[:, h2, :], in_=cgp, func=AF.Sigmoid,
                bias=b2t[:, h2 : h2 + 1], scale=1.0,
            )
        # cgT free layout (b, h2); cg = sig_avg + sig_max
        cgT = workp.tile([128, 2, 2], F32, tag=f"cgT{q}")
        nc.vector.tensor_add(
            cgT[:].rearrange("p b h -> p h b"), sigT[:, :, 0:2], sigT[:, :, 2:4]
        )
        # broadcast per-batch gate rows to all partitions via PE:
        # cgb[p, c] = sum_j mask2t[j, p] * cg_rows[j, c]
        cgr = workp.tile([2, 2, 128], F32, tag=f"cgr{q}")  # [b, h2, cp]
        cgb_ps = psp1.tile([128, C], F32, tag="cgbp", name="cgb_ps")
        for h2 in range(2):
            tpr = psp1.tile([2, 128], F32, tag="tpr")
            nc.tensor.transpose(tpr[:], cgT[:, :, h2], identb[:])
            nc.vector.tensor_copy(cgr[:, h2, :], tpr[:])
            nc.tensor.matmul(
                cgb_ps[:, h2 * 128 : (h2 + 1) * 128],
                lhsT=mask2tb[:], rhs=cgr[:, h2, :],
                start=True, stop=True,
            )
        cgb = workp.tile([128, C], BF16, tag=f"cgb{q}", name=f"cgb{q}")
        nc.vector.tensor_copy(cgb[:], cgb_ps[:])
        return cgb

    # per-pair stat tiles (single writer engine each)
    smax = {q: workp.tile([128, NBLK], BF16, tag=f"smax{q}", name=f"smax{q}")
            for q in range(NPAIR)}
    savg = {q: workp.tile([128, NBLK], F32, tag=f"savg{q}", name=f"savg{q}")
            for q in range(NPAIR)}

    junk = workp.tile([128, C], F32, tag="junk", name="junk")

    def phase2_chunk(q, cgb, k, savg_on_act=False):
        """Big all-bf16 tensor_tensor mult (2x_1p) + bf16 smax reduce +
        fp32 savg sum (DVE reduce, or ACT copy-accum when DVE is the
        bottleneck and ACT has slack)."""
        blk = X[q][:, k * CHUNK : (k + 1) * CHUNK, :]
        cgb_rep = bass.AP(tensor=cgb.tensor, offset=cgb.offset,
                          ap=[cgb.ap[0], [0, CHUNK], cgb.ap[1]])
        nc.vector.tensor_tensor(out=blk, in0=blk, in1=cgb_rep, op=MU.mult)
        nc.vector.tensor_reduce(
            out=smax[q][:, k * CHUNK : (k + 1) * CHUNK], in_=blk,
            axis=mybir.AxisListType.X, op=MU.max,
        )
        if savg_on_act:
            for n in range(k * CHUNK, (k + 1) * CHUNK):
                nc.scalar.activation(
                    out=junk[:], in_=X[q][:, n, :], func=AF.Copy,
                    scale=1.0, accum_out=savg[q][:, n : n + 1],
                )
        else:
            nc.vector.tensor_reduce(
                out=savg[q][:, k * CHUNK : (k + 1) * CHUNK], in_=blk,
                axis=mybir.AxisListType.X, op=MU.add,
            )

    def conv(q, feng):
        nc.gpsimd.dma_start(
            _ap(savg_d, q * ROWS_PAIR, [[NBLK, 128], [1, NBLK]]), savg[q][:]
        )
        nc.gpsimd.dma_start(
            _ap(smax_d, q * ROWS_PAIR, [[NBLK, 128], [1, NBLK]]), smax[q][:]
        )
        s_sb = workp.tile([H, 2, 2, 62], F32, tag=f"ssb{q}")  # [h, ic, b, w+pad]
        if feng is nc.scalar:
            nc.scalar.memzero(s_sb[:])
        else:
            nc.vector.memset(s_sb[:], 0.0)
        for ic, srcd in ((0, savg_d), (1, smax_d)):
            nc.gpsimd.dma_start(
                s_sb[0:H, ic, :, 3 : 3 + W],
                _ap(srcd, q * ROWS_PAIR, [[W, H], [SP, 2], [1, W]]),
            )
        # funnel on whichever of ACT/DVE is idle when this conv runs
        s_sb2 = workp.tile([H, 2, 2, 62], F32, tag=f"ssb2{q}")
        if feng is nc.scalar:
            nc.scalar.copy(s_sb2[:], s_sb[:])
        else:
            nc.vector.tensor_copy(s_sb2[:], s_sb[:])
        conv_ps = psp2.tile([H, 2, W], F32, tag="conv")
        for ic in range(2):
            for dw in range(7):
                j = ic * 7 + dw
                nc.tensor.matmul(
                    conv_ps[:], lhsT=bandsb[:, j, :],
                    rhs=s_sb2[:, ic, :, dw : dw + W],
                    start=(j == 0), stop=(j == 13),
                )
        sg_hw = workp.tile([H, 2, W], F32, tag=f"sghw{q}")
        nc.scalar.activation(
            out=sg_hw[:], in_=conv_ps[:], func=AF.Sigmoid,
            bias=convb[:], scale=1.0,
        )
        nc.gpsimd.dma_start(
            _ap(sg_d, q * ROWS_PAIR, [[W, H], [SP, 2], [1, W]]), sg_hw[:]
        )
        sg = workp.tile([128, NBLK], F32, tag=f"sg{q}", name=f"sg{q}")
        nc.gpsimd.dma_start(
            sg[:], _ap(sg_d, q * ROWS_PAIR, [[NBLK, 128], [1, NBLK]])
        )
        return sg

    def phase4_0_chunk(q, sg, k):
        """pair0: ACT per-block scalar mul (bf16 in, f32 staging out), then
        a full-speed HWDGE out-DMA on the sync queue."""
        for n in range(k * CHUNK, (k + 1) * CHUNK):
            nc.scalar.mul(OS[q][:, n, :], X[q][:, n, :], mul=sg[:, n : n + 1])
        nc.sync.dma_start(
            ov[q, :, k * CHUNK : (k + 1) * CHUNK, :],
            OS[q][:, k * CHUNK : (k + 1) * CHUNK, :],
        )

    def phase4_1_chunk(q, sg, k, on_act=False):
        """pair1: one big DVE tensor_tensor per chunk (bf16 xg x stride-0
        broadcast sg) into the f32 staging tile, then HWDGE out-DMA; the
        tail chunks run on ACT once it finishes pair0."""
        if on_act:
            for n in range(k * CHUNK, (k + 1) * CHUNK):
                nc.scalar.mul(OS[q][:, n, :], X[q][:, n, :],
                              mul=sg[:, n : n + 1])
        else:
            sg_rep = bass.AP(tensor=sg.tensor, offset=sg.offset + k * CHUNK,
                             ap=[sg.ap[0], [1, CHUNK], [0, C]])
            nc.vector.tensor_tensor(
                out=OS[q][:, k * CHUNK : (k + 1) * CHUNK, :],
                in0=X[q][:, k * CHUNK : (k + 1) * CHUNK, :],
                in1=sg_rep, op=MU.mult,
            )
        nc.sync.dma_start(
            ov[q, :, k * CHUNK : (k + 1) * CHUNK, :],
            OS[q][:, k * CHUNK : (k + 1) * CHUNK, :],
        )

    # ---------------- pipeline-ordered emission ----------------
    # DVE runs pair0's phase2 uninterrupted (it gates conv0 -> ACT phase4_0
    # -> out0), then pair1's chmax/phase2/phase4; ACT takes the savg tail of
    # pair0, all of phase4_0, and the last phase4_1 chunks.
    for k in range(NCHUNK):
        phase1_chunk(0, k)
    acc0, chsum0 = phase1_folds(0)
    cgb0 = mlp(0, acc0, chsum0)
    for k in range(NCHUNK):
        phase2_chunk(0, cgb0, k)
    for k in range(NCHUNK):
        phase1_chunk(1, k)
    acc1, chsum1 = phase1_folds(1)
    cgb1 = mlp(1, acc1, chsum1)
    sg0 = conv(0, nc.scalar)
    for k in range(NCHUNK):
        phase2_chunk(1, cgb1, k)
    for k in range(4):
        phase4_0_chunk(0, sg0, k)
    sg1 = conv(1, nc.vector)
    for k in range(4, NCHUNK):
        phase4_0_chunk(0, sg0, k)
    for k in range(NCHUNK):
        phase4_1_chunk(1, sg1, k, on_act=(k >= 5))


def _split_evsem_clears(nc):
    """This walrus build rejects EVENT_SEMAPHORE_RANGE_CLEAR over wide sem
    ranges ("ISA wrong length"); split into clears of <=3 sems."""
    for f in nc.m.functions:
        for blk in f.blocks:
            il = blk.instructions
            for i in range(len(il)):
                inst = il[i]
                if type(inst).__name__ != 'InstISA':
                    continue
                d = inst.ant_dict
                if d is None or 'range_first' not in d or 'range_last' not in d:
                    continue
                first, last = d['range_first'], d['range_last']
                if last - first + 1 <= 3:
                    continue
                si = inst.sync_info
                import copy
                reps = []
                a = first
                while a <= last:
                    b = min(a + 2, last)
                    cl = copy.deepcopy(inst)
                    cl.name = f"I-ws{nc.next_id()}"
                    cd = cl.ant_dict
                    cd['range_first'] = a
                    cd['range_last'] = b
                    reps.append(cl)
                    a = b + 1
                reps[0].sync_info = si
                il[i] = reps[0]
                for j, r in enumerate(reps[1:]):
                    il.insert(i + 1 + j, r)
                break


def _split_waits(nc):
    """Walrus in this toolchain accepts at most ONE sync wait per engine
    instruction; Tile freely emits several.  Split the surplus onto injected
    drain carriers (cloned from native Tile drains so they serialize
    correctly) placed immediately before the instruction -- same engine, so
    per-engine program order and semantics are unchanged."""
    import copy

    proto = {}
    for f in nc.m.functions:
        for blk in f.blocks:
            for inst in blk.instructions:
                if type(inst).__name__ == 'InstDrain' and inst.engine not in proto:
                    proto[inst.engine] = inst
    for f in nc.m.functions:
        for blk in f.blocks:
            il = blk.instructions
            i = 0
            while i < len(il):
                inst = il[i]
                si = inst.sync_info
                if si is None or len(si.on_wait) <= 1:
                    i += 1
                    continue
                waits = list(si.on_wait)
                eng = inst.engine
                for w in waits[:-1]:
                    nop = copy.deepcopy(proto[eng])
                    nop.name = f"I-ws{nc.next_id()}"
                    nop.sync_info = type(si)(on_wait=[w], on_update=[])
                    il.insert(i, nop)
                    i += 1
                inst.sync_info = type(si)(
                    on_wait=[waits[-1]], on_update=list(si.on_update)
                )
                i += 1


_NC = {}


def _get_nc(split=True):
    if split not in _NC:
        nc = bass.Bass()
        with tile.TileContext(nc) as tc:
            _emit(tc)
        if split:
            _split_waits(nc)
            _split_evsem_clears(nc)
        _NC[split] = nc
    return _NC[split]


def _host_inputs(w1, b1, w2, b2, conv_w, conv_b):
    import ml_dtypes
    w1 = np.asarray(w1, np.float32)
    w2 = np.asarray(w2, np.float32)
    w1h = np.ascontiguousarray(w1.reshape(2, 128, HID).transpose(1, 0, 2))
    w1sh = np.ascontiguousarray(w1h / float(SP))
    w2h = np.ascontiguousarray(np.asarray(w2, np.float32).reshape(HID, 2, 128))
    b1c = np.ascontiguousarray(np.asarray(b1, np.float32).reshape(HID, 1))
    b2t = np.ascontiguousarray(np.asarray(b2, np.float32).reshape(2, 128).T)
    cw = np.asarray(conv_w, np.float32).reshape(7, 7, 2)
    bands = np.zeros((H, 14, H), np.float32)
    for ic in range(2):
        for dw in range(7):
            for dh in range(7):
                d = dh - 3  # hs - ho
                v = cw[dh, dw, ic]
                if ic == 0:
                    v = v / float(C)  # savg arrives as a raw channel sum
                if d >= 0:
                    idx = np.arange(0, H - d)
                    bands[idx + d, ic * 7 + dw, idx] = v
                else:
                    idx = np.arange(-d, H)
                    bands[idx + d, ic * 7 + dw, idx] = v
    ident = np.eye(128, dtype=np.float32)
    ident16 = np.eye(128, dtype=ml_dtypes.bfloat16)
    mask2 = np.zeros((128, 2), np.float32)
    mask2[0:64, 0] = 1.0
    mask2[64:128, 1] = 1.0
    mask2t = np.ascontiguousarray(mask2.T)
    mask2b16 = mask2.astype(ml_dtypes.bfloat16)
    convb = np.full((H, 1), np.asarray(conv_b, np.float32).reshape(-1)[0], np.float32)
    return dict(w1h=w1h, w1sh=w1sh, w2h=w2h, b1c=b1c, b2t=b2t,
                bands=bands, ident=ident, ident16=ident16, mask2=mask2b16,
                mask2t=mask2t, convb=convb)


def kernel(x, w1, b1, w2, b2, conv_w, conv_b, _trace=False):
    from concourse.bass_utils import run_bass_kernel_spmd

    nc = _get_nc()
    consts = _host_inputs(w1, b1, w2, b2, conv_w, conv_b)
    xs = np.ascontiguousarray(np.asarray(x, np.float32)).reshape(8, ROWS_CORE, C)
    in_maps = [dict(consts, x=xs[i]) for i in range(N_CORES)]
    res = run_bass_kernel_spmd(nc, in_maps, core_ids=list(range(N_CORES)),
                               trace=_trace)
    out = np.stack([r["out"] for r in res.results])  # [8, 12544, 256]
    out = out.reshape(32, H, W, C)
    if _trace:
        kernel.last_results = res
    return out


# revision 27
# speedup vs baseline: 1.0283x; 1.0283x over previous
"""CBAM kernel for Trainium2, 8-core data-parallel (4 batches per core).

Layout trick: per core the shard is [12544, 256] (4 batches x 3136 spatial x 256ch).
Split into 2 batch-PAIRS of [6272, 256]. Within a pair, flat row r = 49*p + n
(p in [0,128), n in [0,49)) puts batch = p//64 exactly on a 64-partition group
(3136 = 64*49), giving fully contiguous per-partition DMA (50KB runs) and
letting every compute op span all 128 partitions.

v3: bf16 data plane. gpsimd-issued CASTING DMAs convert f32->bf16 on the
way in and bf16->f32 on the way out, so every bulk DVE op runs on 2-byte
data (2x_1p mode) and SBUF traffic halves:
  - in: x lands as bf16 (the only copy held on chip; xg overwrites it).
  - chmax chain + smax reduce + phase4 scalar-mul: all-bf16 DVE ops (2x).
  - phase2: per-block scalar_tensor_tensor fuses xg=x*cg with the savg
    sum accumulation (accum register is fp32, exempt from the 2x rule).
  - chsum: PE bf16 matmuls (1 cyc/row) against a bf16 0/1 mask, two
    256-col blocks per matmul (512 moving cols), fp32 PSUM accumulate.
  - savg stays fp32 (sum accuracy); smax travels bf16 through its DRAM
    shuffle and is cast to f32 on the read-back; conv/MLP stay fp32.
  - savg is a raw channel SUM; the 1/C of the mean is folded into the
    ic=0 rows of the conv band matrices on the host.
  - phase4 pair0 on ACT (overlaps DVE's phase2_1), pair1 on DVE.
  - DMA engine budget: HWDGE (sync/scalar queues) only carries the tiny
    const loads; all bulk traffic is SWDGE via gpsimd (casting).
Rounding cost: |rel err| ~ 4e-3 (bf16 x, bf16 xg, bf16 out) against the
f32 reference -- comfortably inside the 2e-2 gate.
"""

import numpy as np
from contextlib import ExitStack

import concourse.bass as bass
import concourse.tile as tile
from concourse import mybir
from concourse._compat import with_exitstack
from concourse.tile import add_dep_helper

F32 = mybir.dt.float32
BF16 = mybir.dt.bfloat16

C = 256
HID = 16
NPAIR = 2          # batch pairs per core
NBLK = 49          # 256-ch blocks per pair free dim (3136 = 64*49)
CHUNK = 7          # blocks per DMA chunk
NCHUNK = NBLK // CHUNK
ROWS_PAIR = 128 * NBLK   # 6272
ROWS_CORE = NPAIR * ROWS_PAIR  # 12544
H = W = 56
SP = H * W  # 3136
N_CORES = 8

MU = mybir.AluOpType
AF = mybir.ActivationFunctionType


def _ap(handle_ap, offset_elems, dims):
    """Raw AP over a DRAM tensor: dims = [[step, count], ...] in elements."""
    base = handle_ap[tuple([slice(None)] * len(handle_ap.shape))]
    return bass.AP(tensor=base.tensor, offset=base.offset + offset_elems, ap=dims)


@with_exitstack
def _emit(ctx: ExitStack, tc: tile.TileContext):
    nc = tc.nc

    x_d = nc.dram_tensor("x", [ROWS_CORE, C], F32, kind="ExternalInput")
    w1h_d = nc.dram_tensor("w1h", [128, 2, HID], F32, kind="ExternalInput")
    w1sh_d = nc.dram_tensor("w1sh", [128, 2, HID], F32, kind="ExternalInput")
    w2h_d = nc.dram_tensor("w2h", [HID, 2, 128], F32, kind="ExternalInput")
    b1c_d = nc.dram_tensor("b1c", [HID, 1], F32, kind="ExternalInput")
    b2t_d = nc.dram_tensor("b2t", [128, 2], F32, kind="ExternalInput")
    bands_d = nc.dram_tensor("bands", [H, 14, H], F32, kind="ExternalInput")
    ident_d = nc.dram_tensor("ident", [128, 128], F32, kind="ExternalInput")
    ident16_d = nc.dram_tensor("ident16", [128, 128], BF16, kind="ExternalInput")
    mask2_d = nc.dram_tensor("mask2", [128, 2], BF16, kind="ExternalInput")
    mask2t_d = nc.dram_tensor("mask2t", [2, 128], F32, kind="ExternalInput")
    convb_d = nc.dram_tensor("convb", [H, 1], F32, kind="ExternalInput")
    out_d = nc.dram_tensor("out", [ROWS_CORE, C], F32, kind="ExternalOutput")

    # DRAM scratch for the conv-input / spatial-gate reshuffles
    savg_d = nc.dram_tensor("savg_s", [NPAIR, ROWS_PAIR], F32)
    smax_d = nc.dram_tensor("smax_s", [NPAIR, ROWS_PAIR], F32)
    sg_d = nc.dram_tensor("sg_s", [NPAIR, ROWS_PAIR], F32)

    xv = x_d[:, :].rearrange("(q p n) c -> q p n c", q=NPAIR, p=128)
    ov = out_d[:, :].rearrange("(q p n) c -> q p n c", q=NPAIR, p=128)

    constp = ctx.enter_context(tc.tile_pool(name="const", bufs=1))
    bigp = ctx.enter_context(tc.tile_pool(name="big", bufs=1))
    workp = ctx.enter_context(tc.tile_pool(name="work", bufs=1))
    psp1 = ctx.enter_context(tc.tile_pool(name="ps1", bufs=1, space="PSUM"))
    psp2 = ctx.enter_context(tc.tile_pool(name="ps2", bufs=2, space="PSUM"))

    # ---- in-DMA: fp32 on the sync HWDGE queue into a 4-slot ring; the
    # otherwise-idle ACT engine converts each chunk to bf16 (the only copy
    # kept; xg overwrites it). HWDGE starts ~7us and runs the full farm;
    # gpsimd/SWDGE is avoided entirely (its Q7 issue+drain costs dominated
    # earlier traces).
    NSLOT = 4
    X = {}
    OS = {}
    X32r = bigp.tile([128, NSLOT, CHUNK, C], F32, tag="x32r", name="x32r")
    for q in range(NPAIR):
        X[q] = bigp.tile([128, NBLK, C], BF16, tag=f"x{q}", name=f"x{q}")
        OS[q] = bigp.tile([128, NBLK, C], F32, tag=f"os{q}", name=f"os{q}")
    def in_dma(g):
        q, k = divmod(g, NCHUNK)
        nc.sync.dma_start(
            X32r[:, g % NSLOT, :, :],
            xv[q, :, k * CHUNK : (k + 1) * CHUNK, :],
        )

    # prefill the ring; each phase1 chunk's convert then frees a slot and
    # emits the next chunk's DMA (program order keeps the WAR tracking right)
    for g in range(NSLOT):
        in_dma(g)

    # ---- constants via the sync/scalar HWDGE queues (free early) ----
    def const_load(name, shape, dram, dtype=F32, eng=None):
        t = constp.tile(shape, dtype, tag=name, name=name)
        (eng or nc.scalar).dma_start(t[tuple([slice(None)] * len(shape))], dram)
        return t

    # chsum / chmax gates first (PE + transposes touch these earliest)
    mask2 = const_load("mask2", [128, 2], mask2_d[:, :], dtype=BF16)
    ident16 = const_load("ident16", [128, 128], ident16_d[:, :], dtype=BF16)
    ident = const_load("ident", [128, 128], ident_d[:, :])
    w1h = const_load("w1h", [128, 2, HID], w1h_d[:, :, :])
    w1sh = const_load("w1sh", [128, 2, HID], w1sh_d[:, :, :])
    w2h = const_load("w2h", [HID, 2, 128], w2h_d[:, :, :])
    b1c = const_load("b1c", [HID, 1], b1c_d[:, :])
    b2t = const_load("b2t", [128, 2], b2t_d[:, :])
    mask2t = const_load("mask2t", [2, 128], mask2t_d[:, :])
    bands = const_load("bands", [H, 14, H], bands_d[:, :, :])
    convb = const_load("convb", [H, 1], convb_d[:, :])

    # DVE funnel copies: every fp32 matmul operand must reach PE with deps on
    # at most one engine (fused-LDWEIGHTS fp32 matmuls tolerate 1 sync wait).
    def funnel(name, src, shape):
        t = constp.tile(shape, F32, tag=name)
        nc.vector.tensor_copy(t[tuple([slice(None)] * len(shape))],
                              src[tuple([slice(None)] * len(shape))])
        return t

    identb = funnel("identb", ident, [128, 128])
    w1hb = funnel("w1hb", w1h, [128, 2, HID])
    w1shb = funnel("w1shb", w1sh, [128, 2, HID])
    w2hb = funnel("w2hb", w2h, [HID, 2, 128])
    bandsb = funnel("bandsb", bands, [H, 14, H])
    mask2tb = funnel("mask2tb", mask2t, [2, 128])

    prev = {}
    aw = {q: workp.tile([128, CHUNK, C], BF16, tag=f"aw{q}", name=f"aw{q}")
          for q in range(NPAIR)}
    chsum = {}

    def phase1_chunk(q, k):
        """ACT converts the landed fp32 chunk to bf16, then chmax chain on
        DVE (bf16, 2x) + chsum on PE (bf16, two blocks per matmul = 512
        moving cols, fp32 PSUM accumulate)."""
        if k == 0:
            chsum[q] = psp2.tile([2, 2, C], F32, tag="chsum", name="chsum")
        g = q * NCHUNK + k
        blk = X[q][:, k * CHUNK : (k + 1) * CHUNK, :]
        nc.scalar.copy(blk, X32r[:, g % NSLOT, :, :])
        if g + NSLOT < NPAIR * NCHUNK:
            in_dma(g + NSLOT)
        if k == 0:
            nc.vector.tensor_copy(aw[q][:], blk)
        else:
            nc.vector.tensor_max(aw[q][:], aw[q][:], blk)
        n0 = k * CHUNK
        for j in range(3):
            mm = nc.tensor.matmul(
                chsum[q][:], lhsT=mask2[:],
                rhs=X[q][:, n0 + 2 * j : n0 + 2 * j + 2, :],
                start=(k == 0 and j == 0), stop=False,
                skip_group_check=True,
            )
            if k == 0 and j == 0:
                if "last_chsum" in prev:
                    add_dep_helper(mm.ins, prev["last_chsum"].ins, sync=False,
                                   reason="pair order on PE")
        # odd 7th block accumulates into the first half
        mm = nc.tensor.matmul(
            chsum[q][:, 0, :], lhsT=mask2[:], rhs=X[q][:, n0 + 6, :],
            start=False, stop=(k == NCHUNK - 1),
            skip_group_check=True,
        )
        if k == NCHUNK - 1:
            prev["last_chsum"] = mm

    def phase1_folds(q):
        a = aw[q]
        nc.vector.tensor_max(a[:, 0:3, :], a[:, 0:3, :], a[:, 3:6, :])
        nc.vector.tensor_max(a[:, 0, :], a[:, 0, :], a[:, 1, :])
        nc.vector.tensor_max(a[:, 0, :], a[:, 0, :], a[:, 2, :])
        nc.vector.tensor_max(a[:, 0, :], a[:, 0, :], a[:, 6, :])
        return a[:, 0, :], chsum[q]

    def mlp(q, acc, chsum_ps):
        # statsT[c_in_half, half, stat(avg=0,max=1), b]
        statsT = workp.tile([128, 2, 2, 2], F32, tag=f"stats{q}")
        # fold the two psum halves -> channel sums [2, C]
        sum_sb = workp.tile([2, C], F32, tag=f"sum{q}")
        nc.vector.tensor_copy(sum_sb[:], chsum_ps[:, 0, :])
        nc.vector.tensor_add(sum_sb[:], sum_sb[:], chsum_ps[:, 1, :])
        mlp_ps = psp1.tile([128, 16], F32, tag="mlp")
        for h2 in range(2):
            tp = psp1.tile([128, 128], BF16, tag="tp")
            nc.tensor.transpose(tp[:], acc[:, h2 * 128 : (h2 + 1) * 128],
                                ident16[:])
            nc.vector.tensor_reduce(
                out=statsT[:, h2, 1, :],
                in_=tp[:].rearrange("c (b p) -> c b p", b=2),
                axis=mybir.AxisListType.X, op=MU.max,
            )
            nc.tensor.transpose(
                mlp_ps[:, 2 * h2 : 2 * h2 + 2],
                sum_sb[:, h2 * 128 : (h2 + 1) * 128],
                identb[0:2, 0:2],
            )
            nc.vector.tensor_copy(
                statsT[:, h2, 0, :], mlp_ps[:, 2 * h2 : 2 * h2 + 2]
            )

        for stat in range(2):
            w1x = w1shb if stat == 0 else w1hb
            for h2 in range(2):
                nc.tensor.matmul(
                    mlp_ps[0:HID, 4 + 2 * stat : 6 + 2 * stat],
                    lhsT=w1x[:, h2, :], rhs=statsT[:, h2, stat, :],
                    start=(h2 == 0), stop=(h2 == 1),
                )
        h_sb = workp.tile([HID, 2, 2], F32, tag=f"hsb{q}")
        # h = max(h_ps + b1, 0)  (relu on DVE to keep ACT tables stable)
        nc.vector.tensor_scalar(
            out=h_sb[:], in0=mlp_ps[0:HID, 4:8].rearrange("p (s b) -> p s b", s=2),
            scalar1=b1c[:], scalar2=0.0,
            op0=MU.add, op1=MU.max,
        )
        sigT = workp.tile([128, 2, 4], F32, tag=f"sig{q}")
        for h2 in range(2):
            cgp = mlp_ps[:, 8 + 4 * h2 : 12 + 4 * h2]
            nc.tensor.matmul(
                cgp, lhsT=w2hb[:, h2, :], rhs=h_sb[:, :, :],
                start=True, stop=True,
            )
            nc.scalar.activation(
                out=sigT[:, h2, :], in_=cgp, func=AF.Sigmoid,
                bias=b2t[:, h2 : h2 + 1], scale=1.0,
            )
        # cgT free layout (b, h2); cg = sig_avg + sig_max
        cgT = workp.tile([128, 2, 2], F32, tag=f"cgT{q}")
        nc.vector.tensor_add(
            cgT[:].rearrange("p b h -> p h b"), sigT[:, :, 0:2], sigT[:, :, 2:4]
        )
        # broadcast per-batch gate rows to all partitions via PE:
        # cgb[p, c] = sum_j mask2t[j, p] * cg_rows[j, c]
        cgr = workp.tile([2, 2, 128], F32, tag=f"cgr{q}")  # [b, h2, cp]
        cgb_ps = psp1.tile([128, C], F32, tag="cgbp", name="cgb_ps")
        for h2 in range(2):
            tpr = psp1.tile([2, 128], F32, tag="tpr")
            nc.tensor.transpose(tpr[:], cgT[:, :, h2], identb[:])
            nc.vector.tensor_copy(cgr[:, h2, :], tpr[:])
            nc.tensor.matmul(
                cgb_ps[:, h2 * 128 : (h2 + 1) * 128],
                lhsT=mask2tb[:], rhs=cgr[:, h2, :],
                start=True, stop=True,
            )
        cgb = workp.tile([128, C], BF16, tag=f"cgb{q}", name=f"cgb{q}")
        nc.vector.tensor_copy(cgb[:], cgb_ps[:])
        return cgb

    # per-pair stat tiles (single writer engine each)
    smax = {q: workp.tile([128, NBLK], F32, tag=f"smax{q}", name=f"smax{q}")
            for q in range(NPAIR)}
    savg = {q: workp.tile([128, NBLK], F32, tag=f"savg{q}", name=f"savg{q}")
            for q in range(NPAIR)}

    junk = workp.tile([128, C], F32, tag="junk", name="junk")

    def phase2_chunk(q, cgb, k, savg_on_act=False):
        """Big all-bf16 tensor_tensor mult (2x_1p) + bf16 smax reduce +
        fp32 savg sum (DVE reduce, or ACT copy-accum when DVE is the
        bottleneck and ACT has slack)."""
        blk = X[q][:, k * CHUNK : (k + 1) * CHUNK, :]
        for n in range(k * CHUNK, (k + 1) * CHUNK):
            nc.vector.scalar_tensor_tensor(
                out=X[q][:, n, :], in0=X[q][:, n, :], scalar=1.0,
                in1=cgb[:], op0=MU.mult, op1=MU.mult,
                accum_out=savg[q][:, n : n + 1],
            )
        nc.vector.tensor_reduce(
            out=smax[q][:, k * CHUNK : (k + 1) * CHUNK], in_=blk,
            axis=mybir.AxisListType.X, op=MU.max,
        )

    def conv(q, feng):
        nc.scalar.dma_start(
            _ap(savg_d, q * ROWS_PAIR, [[NBLK, 128], [1, NBLK]]), savg[q][:]
        )
        nc.scalar.dma_start(
            _ap(smax_d, q * ROWS_PAIR, [[NBLK, 128], [1, NBLK]]), smax[q][:]
        )
        s_sb = workp.tile([H, 2, 2, 62], F32, tag=f"ssb{q}")  # [h, ic, b, w+pad]
        if feng is nc.scalar:
            nc.scalar.memzero(s_sb[:])
        else:
            nc.vector.memset(s_sb[:], 0.0)
        for ic, srcd in ((0, savg_d), (1, smax_d)):
            nc.scalar.dma_start(
                s_sb[0:H, ic, :, 3 : 3 + W],
                _ap(srcd, q * ROWS_PAIR, [[W, H], [SP, 2], [1, W]]),
            )
        # funnel on whichever of ACT/DVE is idle when this conv runs
        s_sb2 = workp.tile([H, 2, 2, 62], F32, tag=f"ssb2{q}")
        if feng is nc.scalar:
            nc.scalar.copy(s_sb2[:], s_sb[:])
        else:
            nc.vector.tensor_copy(s_sb2[:], s_sb[:])
        conv_ps = psp2.tile([H, 2, W], F32, tag="conv")
        for ic in range(2):
            for dw in range(7):
                j = ic * 7 + dw
                nc.tensor.matmul(
                    conv_ps[:], lhsT=bandsb[:, j, :],
                    rhs=s_sb2[:, ic, :, dw : dw + W],
                    start=(j == 0), stop=(j == 13),
                )
        sg_hw = workp.tile([H, 2, W], F32, tag=f"sghw{q}")
        nc.scalar.activation(
            out=sg_hw[:], in_=conv_ps[:], func=AF.Sigmoid,
            bias=convb[:], scale=1.0,
        )
        nc.scalar.dma_start(
            _ap(sg_d, q * ROWS_PAIR, [[W, H], [SP, 2], [1, W]]), sg_hw[:]
        )
        sg = workp.tile([128, NBLK], F32, tag=f"sg{q}", name=f"sg{q}")
        nc.scalar.dma_start(
            sg[:], _ap(sg_d, q * ROWS_PAIR, [[NBLK, 128], [1, NBLK]])
        )
        return sg

    def phase4_0_chunk(q, sg, k):
        """pair0: ACT per-block scalar mul (bf16 in, f32 staging out), then
        a full-speed HWDGE out-DMA on the sync queue."""
        for n in range(k * CHUNK, (k + 1) * CHUNK):
            nc.scalar.mul(OS[q][:, n, :], X[q][:, n, :], mul=sg[:, n : n + 1])
        nc.sync.dma_start(
            ov[q, :, k * CHUNK : (k + 1) * CHUNK, :],
            OS[q][:, k * CHUNK : (k + 1) * CHUNK, :],
        )

    def phase4_1_chunk(q, sg, k, on_act=False):
        """pair1: one big DVE tensor_tensor per chunk (bf16 xg x stride-0
        broadcast sg) into the f32 staging tile, then HWDGE out-DMA; the
        tail chunks run on ACT once it finishes pair0."""
        if on_act:
            for n in range(k * CHUNK, (k + 1) * CHUNK):
                nc.scalar.mul(OS[q][:, n, :], X[q][:, n, :],
                              mul=sg[:, n : n + 1])
        else:
            sg_rep = bass.AP(tensor=sg.tensor, offset=sg.offset + k * CHUNK,
                             ap=[sg.ap[0], [1, CHUNK], [0, C]])
            nc.vector.tensor_tensor(
                out=OS[q][:, k * CHUNK : (k + 1) * CHUNK, :],
                in0=X[q][:, k * CHUNK : (k + 1) * CHUNK, :],
                in1=sg_rep, op=MU.mult,
            )
        nc.sync.dma_start(
            ov[q, :, k * CHUNK : (k + 1) * CHUNK, :],
            OS[q][:, k * CHUNK : (k + 1) * CHUNK, :],
        )

    # ---------------- pipeline-ordered emission ----------------
    # DVE runs pair0's phase2 uninterrupted (it gates conv0 -> ACT phase4_0
    # -> out0), then pair1's chmax/phase2/phase4; ACT takes the savg tail of
    # pair0, all of phase4_0, and the last phase4_1 chunks.
    for k in range(NCHUNK):
        phase1_chunk(0, k)
    acc0, chsum0 = phase1_folds(0)
    cgb0 = mlp(0, acc0, chsum0)
    for k in range(NCHUNK):
        phase2_chunk(0, cgb0, k)
    for k in range(NCHUNK):
        phase1_chunk(1, k)
    acc1, chsum1 = phase1_folds(1)
    cgb1 = mlp(1, acc1, chsum1)
    sg0 = conv(0, nc.scalar)
    for k in range(NCHUNK):
        phase2_chunk(1, cgb1, k)
    for k in range(4):
        phase4_0_chunk(0, sg0, k)
    sg1 = conv(1, nc.vector)
    for k in range(4, NCHUNK):
        phase4_0_chunk(0, sg0, k)
    for k in range(NCHUNK):
        phase4_1_chunk(1, sg1, k, on_act=(k >= 5))


def _split_evsem_clears(nc):
    """This walrus build rejects EVENT_SEMAPHORE_RANGE_CLEAR over wide sem
    ranges ("ISA wrong length"); split into clears of <=3 sems."""
    for f in nc.m.functions:
        for blk in f.blocks:
            il = blk.instructions
            for i in range(len(il)):
                inst = il[i]
                if type(inst).__name__ != 'InstISA':
                    continue
                d = inst.ant_dict
                if d is None or 'range_first' not in d or 'range_last' not in d:
                    continue
                first, last = d['range_first'], d['range_last']
                if last - first + 1 <= 3:
                    continue
                si = inst.sync_info
                import copy
                reps = []
                a = first
                while a <= last:
                    b = min(a + 2, last)
                    cl = copy.deepcopy(inst)
                    cl.name = f"I-ws{nc.next_id()}"
                    cd = cl.ant_dict
                    cd['range_first'] = a
                    cd['range_last'] = b
                    reps.append(cl)
                    a = b + 1
                reps[0].sync_info = si
                il[i] = reps[0]
                for j, r in enumerate(reps[1:]):
                    il.insert(i + 1 + j, r)
                break


def _split_waits(nc):
    """Walrus in this toolchain accepts at most ONE sync wait per engine
    instruction; Tile freely emits several.  Split the surplus onto injected
    drain carriers (cloned from native Tile drains so they serialize
    correctly) placed immediately before the instruction -- same engine, so
    per-engine program order and semantics are unchanged."""
    import copy

    proto = {}
    for f in nc.m.functions:
        for blk in f.blocks:
            for inst in blk.instructions:
                if type(inst).__name__ == 'InstDrain' and inst.engine not in proto:
                    proto[inst.engine] = inst
    for f in nc.m.functions:
        for blk in f.blocks:
            il = blk.instructions
            i = 0
            while i < len(il):
                inst = il[i]
                si = inst.sync_info
                if si is None or len(si.on_wait) <= 1:
                    i += 1
                    continue
                waits = list(si.on_wait)
                eng = inst.engine
                for w in waits[:-1]:
                    nop = copy.deepcopy(proto[eng])
                    nop.name = f"I-ws{nc.next_id()}"
                    nop.sync_info = type(si)(on_wait=[w], on_update=[])
                    il.insert(i, nop)
                    i += 1
                inst.sync_info = type(si)(
                    on_wait=[waits[-1]], on_update=list(si.on_update)
                )
                i += 1


_NC = {}


def _get_nc(split=True):
    if split not in _NC:
        nc = bass.Bass()
        with tile.TileContext(nc) as tc:
            _emit(tc)
        if split:
            _split_waits(nc)
            _split_evsem_clears(nc)
        _NC[split] = nc
    return _NC[split]


def _host_inputs(w1, b1, w2, b2, conv_w, conv_b):
    import ml_dtypes
    w1 = np.asarray(w1, np.float32)
    w2 = np.asarray(w2, np.float32)
    w1h = np.ascontiguousarray(w1.reshape(2, 128, HID).transpose(1, 0, 2))
    w1sh = np.ascontiguousarray(w1h / float(SP))
    w2h = np.ascontiguousarray(np.asarray(w2, np.float32).reshape(HID, 2, 128))
    b1c = np.ascontiguousarray(np.asarray(b1, np.float32).reshape(HID, 1))
    b2t = np.ascontiguousarray(np.asarray(b2, np.float32).reshape(2, 128).T)
    cw = np.asarray(conv_w, np.float32).reshape(7, 7, 2)
    bands = np.zeros((H, 14, H), np.float32)
    for ic in range(2):
        for dw in range(7):
            for dh in range(7):
                d = dh - 3  # hs - ho
                v = cw[dh, dw, ic]
                if ic == 0:
                    v = v / float(C)  # savg arrives as a raw channel sum
                if d >= 0:
                    idx = np.arange(0, H - d)
                    bands[idx + d, ic * 7 + dw, idx] = v
                else:
                    idx = np.arange(-d, H)
                    bands[idx + d, ic * 7 + dw, idx] = v
    ident = np.eye(128, dtype=np.float32)
    ident16 = np.eye(128, dtype=ml_dtypes.bfloat16)
    mask2 = np.zeros((128, 2), np.float32)
    mask2[0:64, 0] = 1.0
    mask2[64:128, 1] = 1.0
    mask2t = np.ascontiguousarray(mask2.T)
    mask2b16 = mask2.astype(ml_dtypes.bfloat16)
    convb = np.full((H, 1), np.asarray(conv_b, np.float32).reshape(-1)[0], np.float32)
    return dict(w1h=w1h, w1sh=w1sh, w2h=w2h, b1c=b1c, b2t=b2t,
                bands=bands, ident=ident, ident16=ident16, mask2=mask2b16,
                mask2t=mask2t, convb=convb)


def kernel(x, w1, b1, w2, b2, conv_w, conv_b, _trace=False):
    from concourse.bass_utils import run_bass_kernel_spmd

    nc = _get_nc()
    consts = _host_inputs(w1, b1, w2, b2, conv_w, conv_b)
    xs = np.ascontiguousarray(np.asarray(x, np.float32)).reshape(8, ROWS_CORE, C)
    in_maps = [dict(consts, x=xs[i]) for i in range(N_CORES)]
    res = run_bass_kernel_spmd(nc, in_maps, core_ids=list(range(N_CORES)),
                               trace=_trace)
    out = np.stack([r["out"] for r in res.results])  # [8, 12544, 256]
    out = out.reshape(32, H, W, C)
    if _trace:
        kernel.last_results = res
    return out


# revision 29
# speedup vs baseline: 1.1152x; 1.0845x over previous
"""CBAM kernel for Trainium2, 8-core data-parallel (4 batches per core).

Layout trick: per core the shard is [12544, 256] (4 batches x 3136 spatial x 256ch).
Split into 2 batch-PAIRS of [6272, 256]. Within a pair, flat row r = 49*p + n
(p in [0,128), n in [0,49)) puts batch = p//64 exactly on a 64-partition group
(3136 = 64*49), giving fully contiguous per-partition DMA (50KB runs) and
letting every compute op span all 128 partitions.

v3: bf16 data plane. gpsimd-issued CASTING DMAs convert f32->bf16 on the
way in and bf16->f32 on the way out, so every bulk DVE op runs on 2-byte
data (2x_1p mode) and SBUF traffic halves:
  - in: x lands as bf16 (the only copy held on chip; xg overwrites it).
  - chmax chain + smax reduce + phase4 scalar-mul: all-bf16 DVE ops (2x).
  - phase2: per-block scalar_tensor_tensor fuses xg=x*cg with the savg
    sum accumulation (accum register is fp32, exempt from the 2x rule).
  - chsum: PE bf16 matmuls (1 cyc/row) against a bf16 0/1 mask, two
    256-col blocks per matmul (512 moving cols), fp32 PSUM accumulate.
  - savg stays fp32 (sum accuracy); smax travels bf16 through its DRAM
    shuffle and is cast to f32 on the read-back; conv/MLP stay fp32.
  - savg is a raw channel SUM; the 1/C of the mean is folded into the
    ic=0 rows of the conv band matrices on the host.
  - phase4 pair0 on ACT (overlaps DVE's phase2_1), pair1 on DVE.
  - DMA engine budget: HWDGE (sync/scalar queues) only carries the tiny
    const loads; all bulk traffic is SWDGE via gpsimd (casting).
Rounding cost: |rel err| ~ 4e-3 (bf16 x, bf16 xg, bf16 out) against the
f32 reference -- comfortably inside the 2e-2 gate.
"""

import numpy as np
from contextlib import ExitStack

import concourse.bass as bass
import concourse.tile as tile
from concourse import mybir
from concourse._compat import with_exitstack
from concourse.tile import add_dep_helper

F32 = mybir.dt.float32
BF16 = mybir.dt.bfloat16

C = 256
HID = 16
NPAIR = 2          # batch pairs per core
NBLK = 49          # 256-ch blocks per pair free dim (3136 = 64*49)
CHUNK = 7          # blocks per DMA chunk
NCHUNK = NBLK // CHUNK
ROWS_PAIR = 128 * NBLK   # 6272
ROWS_CORE = NPAIR * ROWS_PAIR  # 12544
H = W = 56
SP = H * W  # 3136
N_CORES = 8

MU = mybir.AluOpType
AF = mybir.ActivationFunctionType


def _ap(handle_ap, offset_elems, dims):
    """Raw AP over a DRAM tensor: dims = [[step, count], ...] in elements."""
    base = handle_ap[tuple([slice(None)] * len(handle_ap.shape))]
    return bass.AP(tensor=base.tensor, offset=base.offset + offset_elems, ap=dims)


@with_exitstack
def _emit(ctx: ExitStack, tc: tile.TileContext):
    nc = tc.nc

    x_d = nc.dram_tensor("x", [ROWS_CORE, C], F32, kind="ExternalInput")
    w1h_d = nc.dram_tensor("w1h", [128, 2, HID], F32, kind="ExternalInput")
    w1sh_d = nc.dram_tensor("w1sh", [128, 2, HID], F32, kind="ExternalInput")
    w2h_d = nc.dram_tensor("w2h", [HID, 2, 128], F32, kind="ExternalInput")
    b1c_d = nc.dram_tensor("b1c", [HID, 1], F32, kind="ExternalInput")
    b2t_d = nc.dram_tensor("b2t", [128, 2], F32, kind="ExternalInput")
    bands_d = nc.dram_tensor("bands", [H, 14, H], F32, kind="ExternalInput")
    ident_d = nc.dram_tensor("ident", [128, 128], F32, kind="ExternalInput")
    ident16_d = nc.dram_tensor("ident16", [128, 128], BF16, kind="ExternalInput")
    mask2_d = nc.dram_tensor("mask2", [128, 2], BF16, kind="ExternalInput")
    mask2t_d = nc.dram_tensor("mask2t", [2, 128], F32, kind="ExternalInput")
    convb_d = nc.dram_tensor("convb", [H, 1], F32, kind="ExternalInput")
    out_d = nc.dram_tensor("out", [ROWS_CORE, C], F32, kind="ExternalOutput")

    # DRAM scratch for the conv-input / spatial-gate reshuffles
    savg_d = nc.dram_tensor("savg_s", [NPAIR, ROWS_PAIR], F32)
    smax_d = nc.dram_tensor("smax_s", [NPAIR, ROWS_PAIR], F32)
    sg_d = nc.dram_tensor("sg_s", [NPAIR, ROWS_PAIR], F32)

    xv = x_d[:, :].rearrange("(q p n) c -> q p n c", q=NPAIR, p=128)
    ov = out_d[:, :].rearrange("(q p n) c -> q p n c", q=NPAIR, p=128)

    constp = ctx.enter_context(tc.tile_pool(name="const", bufs=1))
    bigp = ctx.enter_context(tc.tile_pool(name="big", bufs=1))
    workp = ctx.enter_context(tc.tile_pool(name="work", bufs=1))
    psp1 = ctx.enter_context(tc.tile_pool(name="ps1", bufs=1, space="PSUM"))
    psp2 = ctx.enter_context(tc.tile_pool(name="ps2", bufs=2, space="PSUM"))

    # ---- in-DMA: fp32 on the sync HWDGE queue into a 4-slot ring; the
    # otherwise-idle ACT engine converts each chunk to bf16 (the only copy
    # kept; xg overwrites it). HWDGE starts ~7us and runs the full farm;
    # gpsimd/SWDGE is avoided entirely (its Q7 issue+drain costs dominated
    # earlier traces).
    NSLOT = 6
    X = {}
    OS = {}
    X32r = bigp.tile([128, NSLOT, CHUNK, C], F32, tag="x32r", name="x32r")
    OSs = bigp.tile([128, NBLK, C], F32, tag="os", name="os")
    for q in range(NPAIR):
        X[q] = bigp.tile([128, NBLK, C], BF16, tag=f"x{q}", name=f"x{q}")
        OS[q] = OSs  # shared: pair1's writes WAR-chain behind pair0's out-DMAs
    def in_dma(g):
        q, k = divmod(g, NCHUNK)
        nc.sync.dma_start(
            X32r[:, g % NSLOT, :, :],
            xv[q, :, k * CHUNK : (k + 1) * CHUNK, :],
        )

    # ---- constants via the sync/scalar HWDGE queues (free early) ----
    def const_load(name, shape, dram, dtype=F32, eng=None):
        t = constp.tile(shape, dtype, tag=name, name=name)
        (eng or nc.sync).dma_start(t[tuple([slice(None)] * len(shape))], dram)
        return t

    # chsum / chmax gates first (PE + transposes touch these earliest),
    # then the x-ring prefill, then the remaining consts
    mask2 = const_load("mask2", [128, 2], mask2_d[:, :], dtype=BF16)
    ident16 = const_load("ident16", [128, 128], ident16_d[:, :], dtype=BF16)
    ident = const_load("ident", [128, 128], ident_d[:, :])
    for g in range(NSLOT):
        in_dma(g)
    w1h = const_load("w1h", [128, 2, HID], w1h_d[:, :, :])
    w1sh = const_load("w1sh", [128, 2, HID], w1sh_d[:, :, :])
    w2h = const_load("w2h", [HID, 2, 128], w2h_d[:, :, :])
    b1c = const_load("b1c", [HID, 1], b1c_d[:, :])
    b2t = const_load("b2t", [128, 2], b2t_d[:, :])
    mask2t = const_load("mask2t", [2, 128], mask2t_d[:, :])
    bands = const_load("bands", [H, 14, H], bands_d[:, :, :])
    convb = const_load("convb", [H, 1], convb_d[:, :])

    # DVE funnel copies: every fp32 matmul operand must reach PE with deps on
    # at most one engine (fused-LDWEIGHTS fp32 matmuls tolerate 1 sync wait).
    def funnel(name, src, shape):
        t = constp.tile(shape, F32, tag=name)
        nc.vector.tensor_copy(t[tuple([slice(None)] * len(shape))],
                              src[tuple([slice(None)] * len(shape))])
        return t

    identb = funnel("identb", ident, [128, 128])
    w1hb = funnel("w1hb", w1h, [128, 2, HID])
    w1shb = funnel("w1shb", w1sh, [128, 2, HID])
    w2hb = funnel("w2hb", w2h, [HID, 2, 128])
    bandsb = funnel("bandsb", bands, [H, 14, H])
    mask2tb = funnel("mask2tb", mask2t, [2, 128])

    prev = {}
    aw = {q: workp.tile([128, CHUNK, C], BF16, tag=f"aw{q}", name=f"aw{q}")
          for q in range(NPAIR)}
    chsum = {}

    def phase1_chunk(q, k):
        """ACT converts the landed fp32 chunk to bf16, then chmax chain on
        DVE (bf16, 2x) + chsum on PE (bf16, two blocks per matmul = 512
        moving cols, fp32 PSUM accumulate)."""
        if k == 0:
            chsum[q] = psp2.tile([2, 2, C], F32, tag="chsum", name="chsum")
        g = q * NCHUNK + k
        blk = X[q][:, k * CHUNK : (k + 1) * CHUNK, :]
        nc.scalar.copy(blk, X32r[:, g % NSLOT, :, :])
        if g + NSLOT < NPAIR * NCHUNK:
            in_dma(g + NSLOT)
        if k == 0:
            nc.vector.tensor_copy(aw[q][:], blk)
        else:
            nc.vector.tensor_max(aw[q][:], aw[q][:], blk)
        n0 = k * CHUNK
        for j in range(3):
            mm = nc.tensor.matmul(
                chsum[q][:], lhsT=mask2[:],
                rhs=X[q][:, n0 + 2 * j : n0 + 2 * j + 2, :],
                start=(k == 0 and j == 0), stop=False,
                skip_group_check=True,
            )
            if k == 0 and j == 0:
                if "last_chsum" in prev:
                    add_dep_helper(mm.ins, prev["last_chsum"].ins, sync=False,
                                   reason="pair order on PE")
        # odd 7th block accumulates into the first half
        mm = nc.tensor.matmul(
            chsum[q][:, 0, :], lhsT=mask2[:], rhs=X[q][:, n0 + 6, :],
            start=False, stop=(k == NCHUNK - 1),
            skip_group_check=True,
        )
        if k == NCHUNK - 1:
            prev["last_chsum"] = mm

    def phase1_folds(q):
        a = aw[q]
        nc.vector.tensor_max(a[:, 0:3, :], a[:, 0:3, :], a[:, 3:6, :])
        nc.vector.tensor_max(a[:, 0, :], a[:, 0, :], a[:, 1, :])
        nc.vector.tensor_max(a[:, 0, :], a[:, 0, :], a[:, 2, :])
        nc.vector.tensor_max(a[:, 0, :], a[:, 0, :], a[:, 6, :])
        return a[:, 0, :], chsum[q]

    def mlp(q, acc, chsum_ps):
        # statsT[c_in_half, half, stat(avg=0,max=1), b]
        statsT = workp.tile([128, 2, 2, 2], F32, tag=f"stats{q}")
        # fold the two psum halves -> channel sums [2, C]
        sum_sb = workp.tile([2, C], F32, tag=f"sum{q}")
        nc.vector.tensor_copy(sum_sb[:], chsum_ps[:, 0, :])
        nc.vector.tensor_add(sum_sb[:], sum_sb[:], chsum_ps[:, 1, :])
        mlp_ps = psp1.tile([128, 16], F32, tag="mlp")
        for h2 in range(2):
            tp = psp1.tile([128, 128], BF16, tag="tp")
            nc.tensor.transpose(tp[:], acc[:, h2 * 128 : (h2 + 1) * 128],
                                ident16[:])
            nc.vector.tensor_reduce(
                out=statsT[:, h2, 1, :],
                in_=tp[:].rearrange("c (b p) -> c b p", b=2),
                axis=mybir.AxisListType.X, op=MU.max,
            )
            nc.tensor.transpose(
                mlp_ps[:, 2 * h2 : 2 * h2 + 2],
                sum_sb[:, h2 * 128 : (h2 + 1) * 128],
                identb[0:2, 0:2],
            )
            nc.vector.tensor_copy(
                statsT[:, h2, 0, :], mlp_ps[:, 2 * h2 : 2 * h2 + 2]
            )

        for stat in range(2):
            w1x = w1shb if stat == 0 else w1hb
            for h2 in range(2):
                nc.tensor.matmul(
                    mlp_ps[0:HID, 4 + 2 * stat : 6 + 2 * stat],
                    lhsT=w1x[:, h2, :], rhs=statsT[:, h2, stat, :],
                    start=(h2 == 0), stop=(h2 == 1),
                )
        h_sb = workp.tile([HID, 2, 2], F32, tag=f"hsb{q}")
        # h = max(h_ps + b1, 0)  (relu on DVE to keep ACT tables stable)
        nc.vector.tensor_scalar(
            out=h_sb[:], in0=mlp_ps[0:HID, 4:8].rearrange("p (s b) -> p s b", s=2),
            scalar1=b1c[:], scalar2=0.0,
            op0=MU.add, op1=MU.max,
        )
        sigT = workp.tile([128, 2, 4], F32, tag=f"sig{q}")
        for h2 in range(2):
            cgp = mlp_ps[:, 8 + 4 * h2 : 12 + 4 * h2]
            nc.tensor.matmul(
                cgp, lhsT=w2hb[:, h2, :], rhs=h_sb[:, :, :],
                start=True, stop=True,
            )
            nc.scalar.activation(
                out=sigT[:, h2, :], in_=cgp, func=AF.Sigmoid,
                bias=b2t[:, h2 : h2 + 1], scale=1.0,
            )
        # cgT free layout (b, h2); cg = sig_avg + sig_max
        cgT = workp.tile([128, 2, 2], F32, tag=f"cgT{q}")
        nc.vector.tensor_add(
            cgT[:].rearrange("p b h -> p h b"), sigT[:, :, 0:2], sigT[:, :, 2:4]
        )
        # broadcast per-batch gate rows to all partitions via PE:
        # cgb[p, c] = sum_j mask2t[j, p] * cg_rows[j, c]
        cgr = workp.tile([2, 2, 128], F32, tag=f"cgr{q}")  # [b, h2, cp]
        cgb_ps = psp1.tile([128, C], F32, tag="cgbp", name="cgb_ps")
        for h2 in range(2):
            tpr = psp1.tile([2, 128], F32, tag="tpr")
            nc.tensor.transpose(tpr[:], cgT[:, :, h2], identb[:])
            nc.vector.tensor_copy(cgr[:, h2, :], tpr[:])
            nc.tensor.matmul(
                cgb_ps[:, h2 * 128 : (h2 + 1) * 128],
                lhsT=mask2tb[:], rhs=cgr[:, h2, :],
                start=True, stop=True,
            )
        cgb = workp.tile([128, C], BF16, tag=f"cgb{q}", name=f"cgb{q}")
        nc.vector.tensor_copy(cgb[:], cgb_ps[:])
        return cgb

    # per-pair stat tiles (single writer engine each)
    smax = {q: workp.tile([128, NBLK], F32, tag=f"smax{q}", name=f"smax{q}")
            for q in range(NPAIR)}
    savg = {q: workp.tile([128, NBLK], F32, tag=f"savg{q}", name=f"savg{q}")
            for q in range(NPAIR)}

    junk = workp.tile([128, C], F32, tag="junk", name="junk")

    def phase2_chunk(q, cgb, k, savg_on_act=False):
        """Big all-bf16 tensor_tensor mult (2x_1p) + bf16 smax reduce +
        fp32 savg sum (DVE reduce, or ACT copy-accum when DVE is the
        bottleneck and ACT has slack)."""
        blk = X[q][:, k * CHUNK : (k + 1) * CHUNK, :]
        for n in range(k * CHUNK, (k + 1) * CHUNK):
            nc.vector.scalar_tensor_tensor(
                out=X[q][:, n, :], in0=X[q][:, n, :], scalar=1.0,
                in1=cgb[:], op0=MU.mult, op1=MU.mult,
                accum_out=savg[q][:, n : n + 1],
            )
        nc.vector.tensor_reduce(
            out=smax[q][:, k * CHUNK : (k + 1) * CHUNK], in_=blk,
            axis=mybir.AxisListType.X, op=MU.max,
        )

    def conv(q, feng):
        nc.scalar.dma_start(
            _ap(savg_d, q * ROWS_PAIR, [[NBLK, 128], [1, NBLK]]), savg[q][:]
        )
        nc.scalar.dma_start(
            _ap(smax_d, q * ROWS_PAIR, [[NBLK, 128], [1, NBLK]]), smax[q][:]
        )
        s_sb = workp.tile([H, 2, 2, 62], F32, tag=f"ssb{q}")  # [h, ic, b, w+pad]
        if feng is nc.scalar:
            nc.scalar.memzero(s_sb[:])
        else:
            nc.vector.memset(s_sb[:], 0.0)
        for ic, srcd in ((0, savg_d), (1, smax_d)):
            nc.scalar.dma_start(
                s_sb[0:H, ic, :, 3 : 3 + W],
                _ap(srcd, q * ROWS_PAIR, [[W, H], [SP, 2], [1, W]]),
            )
        # funnel on whichever of ACT/DVE is idle when this conv runs
        s_sb2 = workp.tile([H, 2, 2, 62], F32, tag=f"ssb2{q}")
        if feng is nc.scalar:
            nc.scalar.copy(s_sb2[:], s_sb[:])
        else:
            nc.vector.tensor_copy(s_sb2[:], s_sb[:])
        conv_ps = psp2.tile([H, 2, W], F32, tag="conv")
        for ic in range(2):
            for dw in range(7):
                j = ic * 7 + dw
                nc.tensor.matmul(
                    conv_ps[:], lhsT=bandsb[:, j, :],
                    rhs=s_sb2[:, ic, :, dw : dw + W],
                    start=(j == 0), stop=(j == 13),
                )
        sg_hw = workp.tile([H, 2, W], F32, tag=f"sghw{q}")
        nc.scalar.activation(
            out=sg_hw[:], in_=conv_ps[:], func=AF.Sigmoid,
            bias=convb[:], scale=1.0,
        )
        nc.scalar.dma_start(
            _ap(sg_d, q * ROWS_PAIR, [[W, H], [SP, 2], [1, W]]), sg_hw[:]
        )
        sg = workp.tile([128, NBLK], F32, tag=f"sg{q}", name=f"sg{q}")
        nc.scalar.dma_start(
            sg[:], _ap(sg_d, q * ROWS_PAIR, [[NBLK, 128], [1, NBLK]])
        )
        return sg

    def phase4_0_chunk(q, sg, k):
        """pair0: ACT per-block scalar mul (bf16 in, f32 staging out), then
        a full-speed HWDGE out-DMA on the sync queue."""
        for n in range(k * CHUNK, (k + 1) * CHUNK):
            nc.scalar.mul(OS[q][:, n, :], X[q][:, n, :], mul=sg[:, n : n + 1])
        nc.sync.dma_start(
            ov[q, :, k * CHUNK : (k + 1) * CHUNK, :],
            OS[q][:, k * CHUNK : (k + 1) * CHUNK, :],
        )

    def phase4_1_chunk(q, sg, k, on_act=False):
        """pair1: one big DVE tensor_tensor per chunk (bf16 xg x stride-0
        broadcast sg) into the f32 staging tile, then HWDGE out-DMA; the
        tail chunks run on ACT once it finishes pair0."""
        if on_act:
            for n in range(k * CHUNK, (k + 1) * CHUNK):
                nc.scalar.mul(OS[q][:, n, :], X[q][:, n, :],
                              mul=sg[:, n : n + 1])
        else:
            sg_rep = bass.AP(tensor=sg.tensor, offset=sg.offset + k * CHUNK,
                             ap=[sg.ap[0], [1, CHUNK], [0, C]])
            nc.vector.tensor_tensor(
                out=OS[q][:, k * CHUNK : (k + 1) * CHUNK, :],
                in0=X[q][:, k * CHUNK : (k + 1) * CHUNK, :],
                in1=sg_rep, op=MU.mult,
            )
        nc.sync.dma_start(
            ov[q, :, k * CHUNK : (k + 1) * CHUNK, :],
            OS[q][:, k * CHUNK : (k + 1) * CHUNK, :],
        )

    # ---------------- pipeline-ordered emission ----------------
    # DVE runs pair0's phase2 uninterrupted (it gates conv0 -> ACT phase4_0
    # -> out0), then pair1's chmax/phase2/phase4; ACT takes the savg tail of
    # pair0, all of phase4_0, and the last phase4_1 chunks.
    for k in range(NCHUNK):
        phase1_chunk(0, k)
    acc0, chsum0 = phase1_folds(0)
    cgb0 = mlp(0, acc0, chsum0)
    phase2_chunk(0, cgb0, 0)
    phase2_chunk(0, cgb0, 1)
    for k in range(3):
        phase1_chunk(1, k)
    phase2_chunk(0, cgb0, 2)
    phase2_chunk(0, cgb0, 3)
    for k in range(3, NCHUNK):
        phase1_chunk(1, k)
    acc1, chsum1 = phase1_folds(1)
    cgb1 = mlp(1, acc1, chsum1)
    for k in range(4, NCHUNK):
        phase2_chunk(0, cgb0, k)
    sg0 = conv(0, nc.scalar)
    for k in range(NCHUNK):
        phase2_chunk(1, cgb1, k)
    for k in range(4):
        phase4_0_chunk(0, sg0, k)
    sg1 = conv(1, nc.vector)
    for k in range(4, NCHUNK):
        phase4_0_chunk(0, sg0, k)
    for k in range(NCHUNK):
        phase4_1_chunk(1, sg1, k, on_act=(k >= 5))


def _split_evsem_clears(nc):
    """This walrus build rejects EVENT_SEMAPHORE_RANGE_CLEAR over wide sem
    ranges ("ISA wrong length"); split into clears of <=3 sems."""
    for f in nc.m.functions:
        for blk in f.blocks:
            il = blk.instructions
            for i in range(len(il)):
                inst = il[i]
                if type(inst).__name__ != 'InstISA':
                    continue
                d = inst.ant_dict
                if d is None or 'range_first' not in d or 'range_last' not in d:
                    continue
                first, last = d['range_first'], d['range_last']
                if last - first + 1 <= 3:
                    continue
                si = inst.sync_info
                import copy
                reps = []
                a = first
                while a <= last:
                    b = min(a + 2, last)
                    cl = copy.deepcopy(inst)
                    cl.name = f"I-ws{nc.next_id()}"
                    cd = cl.ant_dict
                    cd['range_first'] = a
                    cd['range_last'] = b
                    reps.append(cl)
                    a = b + 1
                reps[0].sync_info = si
                il[i] = reps[0]
                for j, r in enumerate(reps[1:]):
                    il.insert(i + 1 + j, r)
                break


def _split_waits(nc):
    """Walrus in this toolchain accepts at most ONE sync wait per engine
    instruction; Tile freely emits several.  Split the surplus onto injected
    drain carriers (cloned from native Tile drains so they serialize
    correctly) placed immediately before the instruction -- same engine, so
    per-engine program order and semantics are unchanged."""
    import copy

    proto = {}
    for f in nc.m.functions:
        for blk in f.blocks:
            for inst in blk.instructions:
                if type(inst).__name__ == 'InstDrain' and inst.engine not in proto:
                    proto[inst.engine] = inst
    for f in nc.m.functions:
        for blk in f.blocks:
            il = blk.instructions
            i = 0
            while i < len(il):
                inst = il[i]
                si = inst.sync_info
                if si is None or len(si.on_wait) <= 1:
                    i += 1
                    continue
                waits = list(si.on_wait)
                eng = inst.engine
                for w in waits[:-1]:
                    nop = copy.deepcopy(proto[eng])
                    nop.name = f"I-ws{nc.next_id()}"
                    nop.sync_info = type(si)(on_wait=[w], on_update=[])
                    il.insert(i, nop)
                    i += 1
                inst.sync_info = type(si)(
                    on_wait=[waits[-1]], on_update=list(si.on_update)
                )
                i += 1


_NC = {}


def _get_nc(split=True):
    if split not in _NC:
        nc = bass.Bass()
        with tile.TileContext(nc) as tc:
            _emit(tc)
        if split:
            _split_waits(nc)
            _split_evsem_clears(nc)
        _NC[split] = nc
    return _NC[split]


def _host_inputs(w1, b1, w2, b2, conv_w, conv_b):
    import ml_dtypes
    w1 = np.asarray(w1, np.float32)
    w2 = np.asarray(w2, np.float32)
    w1h = np.ascontiguousarray(w1.reshape(2, 128, HID).transpose(1, 0, 2))
    w1sh = np.ascontiguousarray(w1h / float(SP))
    w2h = np.ascontiguousarray(np.asarray(w2, np.float32).reshape(HID, 2, 128))
    b1c = np.ascontiguousarray(np.asarray(b1, np.float32).reshape(HID, 1))
    b2t = np.ascontiguousarray(np.asarray(b2, np.float32).reshape(2, 128).T)
    cw = np.asarray(conv_w, np.float32).reshape(7, 7, 2)
    bands = np.zeros((H, 14, H), np.float32)
    for ic in range(2):
        for dw in range(7):
            for dh in range(7):
                d = dh - 3  # hs - ho
                v = cw[dh, dw, ic]
                if ic == 0:
                    v = v / float(C)  # savg arrives as a raw channel sum
                if d >= 0:
                    idx = np.arange(0, H - d)
                    bands[idx + d, ic * 7 + dw, idx] = v
                else:
                    idx = np.arange(-d, H)
                    bands[idx + d, ic * 7 + dw, idx] = v
    ident = np.eye(128, dtype=np.float32)
    ident16 = np.eye(128, dtype=ml_dtypes.bfloat16)
    mask2 = np.zeros((128, 2), np.float32)
    mask2[0:64, 0] = 1.0
    mask2[64:128, 1] = 1.0
    mask2t = np.ascontiguousarray(mask2.T)
    mask2b16 = mask2.astype(ml_dtypes.bfloat16)
    convb = np.full((H, 1), np.asarray(conv_b, np.float32).reshape(-1)[0], np.float32)
    return dict(w1h=w1h, w1sh=w1sh, w2h=w2h, b1c=b1c, b2t=b2t,
                bands=bands, ident=ident, ident16=ident16, mask2=mask2b16,
                mask2t=mask2t, convb=convb)


def kernel(x, w1, b1, w2, b2, conv_w, conv_b, _trace=False):
    from concourse.bass_utils import run_bass_kernel_spmd

    nc = _get_nc()
    consts = _host_inputs(w1, b1, w2, b2, conv_w, conv_b)
    xs = np.ascontiguousarray(np.asarray(x, np.float32)).reshape(8, ROWS_CORE, C)
    in_maps = [dict(consts, x=xs[i]) for i in range(N_CORES)]
    res = run_bass_kernel_spmd(nc, in_maps, core_ids=list(range(N_CORES)),
                               trace=_trace)
    out = np.stack([r["out"] for r in res.results])  # [8, 12544, 256]
    out = out.reshape(32, H, W, C)
    if _trace:
        kernel.last_results = res
    return out


# revision 30
# speedup vs baseline: 1.1208x; 1.0051x over previous
"""CBAM kernel for Trainium2, 8-core data-parallel (4 batches per core).

Layout trick: per core the shard is [12544, 256] (4 batches x 3136 spatial x 256ch).
Split into 2 batch-PAIRS of [6272, 256]. Within a pair, flat row r = 49*p + n
(p in [0,128), n in [0,49)) puts batch = p//64 exactly on a 64-partition group
(3136 = 64*49), giving fully contiguous per-partition DMA (50KB runs) and
letting every compute op span all 128 partitions.

v3: bf16 data plane. gpsimd-issued CASTING DMAs convert f32->bf16 on the
way in and bf16->f32 on the way out, so every bulk DVE op runs on 2-byte
data (2x_1p mode) and SBUF traffic halves:
  - in: x lands as bf16 (the only copy held on chip; xg overwrites it).
  - chmax chain + smax reduce + phase4 scalar-mul: all-bf16 DVE ops (2x).
  - phase2: per-block scalar_tensor_tensor fuses xg=x*cg with the savg
    sum accumulation (accum register is fp32, exempt from the 2x rule).
  - chsum: PE bf16 matmuls (1 cyc/row) against a bf16 0/1 mask, two
    256-col blocks per matmul (512 moving cols), fp32 PSUM accumulate.
  - savg stays fp32 (sum accuracy); smax travels bf16 through its DRAM
    shuffle and is cast to f32 on the read-back; conv/MLP stay fp32.
  - savg is a raw channel SUM; the 1/C of the mean is folded into the
    ic=0 rows of the conv band matrices on the host.
  - phase4 pair0 on ACT (overlaps DVE's phase2_1), pair1 on DVE.
  - DMA engine budget: HWDGE (sync/scalar queues) only carries the tiny
    const loads; all bulk traffic is SWDGE via gpsimd (casting).
Rounding cost: |rel err| ~ 4e-3 (bf16 x, bf16 xg, bf16 out) against the
f32 reference -- comfortably inside the 2e-2 gate.
"""

import numpy as np
from contextlib import ExitStack

import concourse.bass as bass
import concourse.tile as tile
from concourse import mybir
from concourse._compat import with_exitstack
from concourse.tile import add_dep_helper

F32 = mybir.dt.float32
BF16 = mybir.dt.bfloat16

C = 256
HID = 16
NPAIR = 2          # batch pairs per core
NBLK = 49          # 256-ch blocks per pair free dim (3136 = 64*49)
CHUNK = 7          # blocks per DMA chunk
NCHUNK = NBLK // CHUNK
ROWS_PAIR = 128 * NBLK   # 6272
ROWS_CORE = NPAIR * ROWS_PAIR  # 12544
H = W = 56
SP = H * W  # 3136
N_CORES = 8

MU = mybir.AluOpType
AF = mybir.ActivationFunctionType


def _ap(handle_ap, offset_elems, dims):
    """Raw AP over a DRAM tensor: dims = [[step, count], ...] in elements."""
    base = handle_ap[tuple([slice(None)] * len(handle_ap.shape))]
    return bass.AP(tensor=base.tensor, offset=base.offset + offset_elems, ap=dims)


@with_exitstack
def _emit(ctx: ExitStack, tc: tile.TileContext):
    nc = tc.nc

    x_d = nc.dram_tensor("x", [ROWS_CORE, C], F32, kind="ExternalInput")
    w1h_d = nc.dram_tensor("w1h", [128, 2, HID], F32, kind="ExternalInput")
    w1sh_d = nc.dram_tensor("w1sh", [128, 2, HID], F32, kind="ExternalInput")
    w2h_d = nc.dram_tensor("w2h", [HID, 2, 128], F32, kind="ExternalInput")
    b1c_d = nc.dram_tensor("b1c", [HID, 1], F32, kind="ExternalInput")
    b2t_d = nc.dram_tensor("b2t", [128, 2], F32, kind="ExternalInput")
    bands_d = nc.dram_tensor("bands", [H, 14, H], F32, kind="ExternalInput")
    ident_d = nc.dram_tensor("ident", [128, 128], F32, kind="ExternalInput")
    ident16_d = nc.dram_tensor("ident16", [128, 128], BF16, kind="ExternalInput")
    mask2_d = nc.dram_tensor("mask2", [128, 2], BF16, kind="ExternalInput")
    mask2t_d = nc.dram_tensor("mask2t", [2, 128], F32, kind="ExternalInput")
    convb_d = nc.dram_tensor("convb", [H, 1], F32, kind="ExternalInput")
    out_d = nc.dram_tensor("out", [ROWS_CORE, C], F32, kind="ExternalOutput")

    # DRAM scratch for the conv-input / spatial-gate reshuffles
    savg_d = nc.dram_tensor("savg_s", [NPAIR, ROWS_PAIR], F32)
    smax_d = nc.dram_tensor("smax_s", [NPAIR, ROWS_PAIR], F32)
    sg_d = nc.dram_tensor("sg_s", [NPAIR, ROWS_PAIR], F32)

    xv = x_d[:, :].rearrange("(q p n) c -> q p n c", q=NPAIR, p=128)
    ov = out_d[:, :].rearrange("(q p n) c -> q p n c", q=NPAIR, p=128)

    constp = ctx.enter_context(tc.tile_pool(name="const", bufs=1))
    bigp = ctx.enter_context(tc.tile_pool(name="big", bufs=1))
    workp = ctx.enter_context(tc.tile_pool(name="work", bufs=1))
    psp1 = ctx.enter_context(tc.tile_pool(name="ps1", bufs=1, space="PSUM"))
    psp2 = ctx.enter_context(tc.tile_pool(name="ps2", bufs=2, space="PSUM"))

    # ---- in-DMA: fp32 on the sync HWDGE queue into a 4-slot ring; the
    # otherwise-idle ACT engine converts each chunk to bf16 (the only copy
    # kept; xg overwrites it). HWDGE starts ~7us and runs the full farm;
    # gpsimd/SWDGE is avoided entirely (its Q7 issue+drain costs dominated
    # earlier traces).
    NSLOT = 6
    X = {}
    OS = {}
    X32r = bigp.tile([128, NSLOT, CHUNK, C], F32, tag="x32r", name="x32r")
    OSs = bigp.tile([128, NBLK, C], F32, tag="os", name="os")
    for q in range(NPAIR):
        X[q] = bigp.tile([128, NBLK, C], BF16, tag=f"x{q}", name=f"x{q}")
        OS[q] = OSs  # pair0 stages here; pair1 stages in the X32 ring
    def in_dma(g):
        q, k = divmod(g, NCHUNK)
        nc.sync.dma_start(
            X32r[:, g % NSLOT, :, :],
            xv[q, :, k * CHUNK : (k + 1) * CHUNK, :],
        )

    # ---- constants via the sync/scalar HWDGE queues (free early) ----
    def const_load(name, shape, dram, dtype=F32, eng=None):
        t = constp.tile(shape, dtype, tag=name, name=name)
        (eng or nc.sync).dma_start(t[tuple([slice(None)] * len(shape))], dram)
        return t

    # chsum / chmax gates first (PE + transposes touch these earliest),
    # then the x-ring prefill, then the remaining consts
    mask2 = const_load("mask2", [128, 2], mask2_d[:, :], dtype=BF16)
    ident16 = const_load("ident16", [128, 128], ident16_d[:, :], dtype=BF16)
    ident = const_load("ident", [128, 128], ident_d[:, :])
    for g in range(NSLOT):
        in_dma(g)
    w1h = const_load("w1h", [128, 2, HID], w1h_d[:, :, :])
    w1sh = const_load("w1sh", [128, 2, HID], w1sh_d[:, :, :])
    w2h = const_load("w2h", [HID, 2, 128], w2h_d[:, :, :])
    b1c = const_load("b1c", [HID, 1], b1c_d[:, :])
    b2t = const_load("b2t", [128, 2], b2t_d[:, :])
    mask2t = const_load("mask2t", [2, 128], mask2t_d[:, :])
    bands = const_load("bands", [H, 14, H], bands_d[:, :, :])
    convb = const_load("convb", [H, 1], convb_d[:, :])

    # DVE funnel copies: every fp32 matmul operand must reach PE with deps on
    # at most one engine (fused-LDWEIGHTS fp32 matmuls tolerate 1 sync wait).
    def funnel(name, src, shape):
        t = constp.tile(shape, F32, tag=name)
        nc.vector.tensor_copy(t[tuple([slice(None)] * len(shape))],
                              src[tuple([slice(None)] * len(shape))])
        return t

    identb = funnel("identb", ident, [128, 128])
    w1hb = funnel("w1hb", w1h, [128, 2, HID])
    w1shb = funnel("w1shb", w1sh, [128, 2, HID])
    w2hb = funnel("w2hb", w2h, [HID, 2, 128])
    bandsb = funnel("bandsb", bands, [H, 14, H])
    mask2tb = funnel("mask2tb", mask2t, [2, 128])

    prev = {}
    aw = {q: workp.tile([128, CHUNK, C], BF16, tag=f"aw{q}", name=f"aw{q}")
          for q in range(NPAIR)}
    chsum = {}

    def phase1_chunk(q, k):
        """ACT converts the landed fp32 chunk to bf16, then chmax chain on
        DVE (bf16, 2x) + chsum on PE (bf16, two blocks per matmul = 512
        moving cols, fp32 PSUM accumulate)."""
        if k == 0:
            chsum[q] = psp2.tile([2, 2, C], F32, tag="chsum", name="chsum")
        g = q * NCHUNK + k
        blk = X[q][:, k * CHUNK : (k + 1) * CHUNK, :]
        cv = nc.scalar.copy(blk, X32r[:, g % NSLOT, :, :])
        if q == 1 and "sig0" in prev:
            # keep pair0's gate sigmoid ahead of pair1's converts in the
            # ACT stream (the scheduler would otherwise queue it behind)
            add_dep_helper(cv.ins, prev["sig0"].ins, sync=False,
                           reason="sig0 before pair1 converts")
        if g + NSLOT < NPAIR * NCHUNK:
            in_dma(g + NSLOT)
        if k == 0:
            nc.vector.tensor_copy(aw[q][:], blk)
        else:
            nc.vector.tensor_max(aw[q][:], aw[q][:], blk)
        n0 = k * CHUNK
        for j in range(3):
            mm = nc.tensor.matmul(
                chsum[q][:], lhsT=mask2[:],
                rhs=X[q][:, n0 + 2 * j : n0 + 2 * j + 2, :],
                start=(k == 0 and j == 0), stop=False,
                skip_group_check=True,
            )
            if k == 0 and j == 0:
                if "last_chsum" in prev:
                    add_dep_helper(mm.ins, prev["last_chsum"].ins, sync=False,
                                   reason="pair order on PE")
        # odd 7th block accumulates into the first half
        mm = nc.tensor.matmul(
            chsum[q][:, 0, :], lhsT=mask2[:], rhs=X[q][:, n0 + 6, :],
            start=False, stop=(k == NCHUNK - 1),
            skip_group_check=True,
        )
        if k == NCHUNK - 1:
            prev["last_chsum"] = mm

    def phase1_folds(q):
        a = aw[q]
        nc.vector.tensor_max(a[:, 0:3, :], a[:, 0:3, :], a[:, 3:6, :])
        nc.vector.tensor_max(a[:, 0, :], a[:, 0, :], a[:, 1, :])
        nc.vector.tensor_max(a[:, 0, :], a[:, 0, :], a[:, 2, :])
        nc.vector.tensor_max(a[:, 0, :], a[:, 0, :], a[:, 6, :])
        return a[:, 0, :], chsum[q]

    def mlp(q, acc, chsum_ps):
        # statsT[c_in_half, half, stat(avg=0,max=1), b]
        statsT = workp.tile([128, 2, 2, 2], F32, tag=f"stats{q}")
        # fold the two psum halves -> channel sums [2, C]
        sum_sb = workp.tile([2, C], F32, tag=f"sum{q}")
        nc.vector.tensor_copy(sum_sb[:], chsum_ps[:, 0, :])
        nc.vector.tensor_add(sum_sb[:], sum_sb[:], chsum_ps[:, 1, :])
        mlp_ps = psp1.tile([128, 16], F32, tag="mlp")
        for h2 in range(2):
            tp = psp1.tile([128, 128], BF16, tag="tp")
            nc.tensor.transpose(tp[:], acc[:, h2 * 128 : (h2 + 1) * 128],
                                ident16[:])
            nc.vector.tensor_reduce(
                out=statsT[:, h2, 1, :],
                in_=tp[:].rearrange("c (b p) -> c b p", b=2),
                axis=mybir.AxisListType.X, op=MU.max,
            )
            nc.tensor.transpose(
                mlp_ps[:, 2 * h2 : 2 * h2 + 2],
                sum_sb[:, h2 * 128 : (h2 + 1) * 128],
                identb[0:2, 0:2],
            )
            nc.vector.tensor_copy(
                statsT[:, h2, 0, :], mlp_ps[:, 2 * h2 : 2 * h2 + 2]
            )

        for stat in range(2):
            w1x = w1shb if stat == 0 else w1hb
            for h2 in range(2):
                nc.tensor.matmul(
                    mlp_ps[0:HID, 4 + 2 * stat : 6 + 2 * stat],
                    lhsT=w1x[:, h2, :], rhs=statsT[:, h2, stat, :],
                    start=(h2 == 0), stop=(h2 == 1),
                )
        h_sb = workp.tile([HID, 2, 2], F32, tag=f"hsb{q}")
        # h = max(h_ps + b1, 0)  (relu on DVE to keep ACT tables stable)
        nc.vector.tensor_scalar(
            out=h_sb[:], in0=mlp_ps[0:HID, 4:8].rearrange("p (s b) -> p s b", s=2),
            scalar1=b1c[:], scalar2=0.0,
            op0=MU.add, op1=MU.max,
        )
        sigT = workp.tile([128, 2, 4], F32, tag=f"sig{q}")
        for h2 in range(2):
            cgp = mlp_ps[:, 8 + 4 * h2 : 12 + 4 * h2]
            nc.tensor.matmul(
                cgp, lhsT=w2hb[:, h2, :], rhs=h_sb[:, :, :],
                start=True, stop=True,
            )
            sgi = nc.scalar.activation(
                out=sigT[:, h2, :], in_=cgp, func=AF.Sigmoid,
                bias=b2t[:, h2 : h2 + 1], scale=1.0,
            )
            prev[f"sig{q}"] = sgi
        # cgT free layout (b, h2); cg = sig_avg + sig_max
        cgT = workp.tile([128, 2, 2], F32, tag=f"cgT{q}")
        nc.vector.tensor_add(
            cgT[:].rearrange("p b h -> p h b"), sigT[:, :, 0:2], sigT[:, :, 2:4]
        )
        # broadcast per-batch gate rows to all partitions via PE:
        # cgb[p, c] = sum_j mask2t[j, p] * cg_rows[j, c]
        cgr = workp.tile([2, 2, 128], F32, tag=f"cgr{q}")  # [b, h2, cp]
        cgb_ps = psp1.tile([128, C], F32, tag="cgbp", name="cgb_ps")
        for h2 in range(2):
            tpr = psp1.tile([2, 128], F32, tag="tpr")
            nc.tensor.transpose(tpr[:], cgT[:, :, h2], identb[:])
            nc.vector.tensor_copy(cgr[:, h2, :], tpr[:])
            nc.tensor.matmul(
                cgb_ps[:, h2 * 128 : (h2 + 1) * 128],
                lhsT=mask2tb[:], rhs=cgr[:, h2, :],
                start=True, stop=True,
            )
        cgb = workp.tile([128, C], BF16, tag=f"cgb{q}", name=f"cgb{q}")
        nc.vector.tensor_copy(cgb[:], cgb_ps[:])
        return cgb

    # per-pair stat tiles (single writer engine each)
    smax = {q: workp.tile([128, NBLK], F32, tag=f"smax{q}", name=f"smax{q}")
            for q in range(NPAIR)}
    savg = {q: workp.tile([128, NBLK], F32, tag=f"savg{q}", name=f"savg{q}")
            for q in range(NPAIR)}

    junk = workp.tile([128, C], F32, tag="junk", name="junk")

    def phase2_chunk(q, cgb, k, savg_on_act=False):
        """Big all-bf16 tensor_tensor mult (2x_1p) + bf16 smax reduce +
        fp32 savg sum (DVE reduce, or ACT copy-accum when DVE is the
        bottleneck and ACT has slack)."""
        blk = X[q][:, k * CHUNK : (k + 1) * CHUNK, :]
        for n in range(k * CHUNK, (k + 1) * CHUNK):
            nc.vector.scalar_tensor_tensor(
                out=X[q][:, n, :], in0=X[q][:, n, :], scalar=1.0,
                in1=cgb[:], op0=MU.mult, op1=MU.mult,
                accum_out=savg[q][:, n : n + 1],
            )
        nc.vector.tensor_reduce(
            out=smax[q][:, k * CHUNK : (k + 1) * CHUNK], in_=blk,
            axis=mybir.AxisListType.X, op=MU.max,
        )

    def conv(q, feng):
        nc.scalar.dma_start(
            _ap(savg_d, q * ROWS_PAIR, [[NBLK, 128], [1, NBLK]]), savg[q][:]
        )
        nc.scalar.dma_start(
            _ap(smax_d, q * ROWS_PAIR, [[NBLK, 128], [1, NBLK]]), smax[q][:]
        )
        s_sb = workp.tile([H, 2, 2, 62], F32, tag=f"ssb{q}")  # [h, ic, b, w+pad]
        if feng is nc.scalar:
            nc.scalar.memzero(s_sb[:])
        else:
            nc.vector.memset(s_sb[:], 0.0)
        for ic, srcd in ((0, savg_d), (1, smax_d)):
            nc.scalar.dma_start(
                s_sb[0:H, ic, :, 3 : 3 + W],
                _ap(srcd, q * ROWS_PAIR, [[W, H], [SP, 2], [1, W]]),
            )
        # funnel on whichever of ACT/DVE is idle when this conv runs
        s_sb2 = workp.tile([H, 2, 2, 62], F32, tag=f"ssb2{q}")
        if feng is nc.scalar:
            nc.scalar.copy(s_sb2[:], s_sb[:])
        else:
            nc.vector.tensor_copy(s_sb2[:], s_sb[:])
        conv_ps = psp2.tile([H, 2, W], F32, tag="conv")
        for ic in range(2):
            for dw in range(7):
                j = ic * 7 + dw
                nc.tensor.matmul(
                    conv_ps[:], lhsT=bandsb[:, j, :],
                    rhs=s_sb2[:, ic, :, dw : dw + W],
                    start=(j == 0), stop=(j == 13),
                )
        sg_hw = workp.tile([H, 2, W], F32, tag=f"sghw{q}")
        nc.scalar.activation(
            out=sg_hw[:], in_=conv_ps[:], func=AF.Sigmoid,
            bias=convb[:], scale=1.0,
        )
        nc.scalar.dma_start(
            _ap(sg_d, q * ROWS_PAIR, [[W, H], [SP, 2], [1, W]]), sg_hw[:]
        )
        sg = workp.tile([128, NBLK], F32, tag=f"sg{q}", name=f"sg{q}")
        nc.scalar.dma_start(
            sg[:], _ap(sg_d, q * ROWS_PAIR, [[NBLK, 128], [1, NBLK]])
        )
        return sg

    def phase4_0_chunk(q, sg, k):
        """pair0: ACT per-block scalar mul (bf16 in, f32 staging out), then
        a full-speed HWDGE out-DMA on the sync queue."""
        for n in range(k * CHUNK, (k + 1) * CHUNK):
            nc.scalar.mul(OS[q][:, n, :], X[q][:, n, :], mul=sg[:, n : n + 1])
        nc.sync.dma_start(
            ov[q, :, k * CHUNK : (k + 1) * CHUNK, :],
            OS[q][:, k * CHUNK : (k + 1) * CHUNK, :],
        )

    def phase4_1_chunk(q, sg, k, on_act=False):
        """pair1: one big DVE tensor_tensor per chunk (bf16 xg x stride-0
        broadcast sg) into the f32 staging tile, then HWDGE out-DMA; the
        tail chunks run on ACT once it finishes pair0."""
        stage = X32r[:, k % NSLOT, :, :]
        if on_act:
            for j in range(CHUNK):
                n = k * CHUNK + j
                nc.scalar.mul(stage[:, j, :], X[q][:, n, :],
                              mul=sg[:, n : n + 1])
        else:
            sg_rep = bass.AP(tensor=sg.tensor, offset=sg.offset + k * CHUNK,
                             ap=[sg.ap[0], [1, CHUNK], [0, C]])
            nc.vector.tensor_tensor(
                out=stage,
                in0=X[q][:, k * CHUNK : (k + 1) * CHUNK, :],
                in1=sg_rep, op=MU.mult,
            )
        nc.sync.dma_start(
            ov[q, :, k * CHUNK : (k + 1) * CHUNK, :], stage,
        )

    # ---------------- pipeline-ordered emission ----------------
    # DVE runs pair0's phase2 uninterrupted (it gates conv0 -> ACT phase4_0
    # -> out0), then pair1's chmax/phase2/phase4; ACT takes the savg tail of
    # pair0, all of phase4_0, and the last phase4_1 chunks.
    for k in range(NCHUNK):
        phase1_chunk(0, k)
    acc0, chsum0 = phase1_folds(0)
    cgb0 = mlp(0, acc0, chsum0)
    phase2_chunk(0, cgb0, 0)
    phase2_chunk(0, cgb0, 1)
    for k in range(3):
        phase1_chunk(1, k)
    phase2_chunk(0, cgb0, 2)
    phase2_chunk(0, cgb0, 3)
    for k in range(3, NCHUNK):
        phase1_chunk(1, k)
    acc1, chsum1 = phase1_folds(1)
    cgb1 = mlp(1, acc1, chsum1)
    for k in range(4, NCHUNK):
        phase2_chunk(0, cgb0, k)
    sg0 = conv(0, nc.scalar)
    for k in range(NCHUNK):
        phase2_chunk(1, cgb1, k)
    for k in range(4):
        phase4_0_chunk(0, sg0, k)
    sg1 = conv(1, nc.vector)
    for k in range(4, NCHUNK):
        phase4_0_chunk(0, sg0, k)
    for k in range(NCHUNK):
        phase4_1_chunk(1, sg1, k, on_act=(k >= 5))


def _split_evsem_clears(nc):
    """This walrus build rejects EVENT_SEMAPHORE_RANGE_CLEAR over wide sem
    ranges ("ISA wrong length"); split into clears of <=3 sems."""
    for f in nc.m.functions:
        for blk in f.blocks:
            il = blk.instructions
            for i in range(len(il)):
                inst = il[i]
                if type(inst).__name__ != 'InstISA':
                    continue
                d = inst.ant_dict
                if d is None or 'range_first' not in d or 'range_last' not in d:
                    continue
                first, last = d['range_first'], d['range_last']
                if last - first + 1 <= 3:
                    continue
                si = inst.sync_info
                import copy
                reps = []
                a = first
                while a <= last:
                    b = min(a + 2, last)
                    cl = copy.deepcopy(inst)
                    cl.name = f"I-ws{nc.next_id()}"
                    cd = cl.ant_dict
                    cd['range_first'] = a
                    cd['range_last'] = b
                    reps.append(cl)
                    a = b + 1
                reps[0].sync_info = si
                il[i] = reps[0]
                for j, r in enumerate(reps[1:]):
                    il.insert(i + 1 + j, r)
                break


def _split_waits(nc):
    """Walrus in this toolchain accepts at most ONE sync wait per engine
    instruction; Tile freely emits several.  Split the surplus onto injected
    drain carriers (cloned from native Tile drains so they serialize
    correctly) placed immediately before the instruction -- same engine, so
    per-engine program order and semantics are unchanged."""
    import copy

    proto = {}
    for f in nc.m.functions:
        for blk in f.blocks:
            for inst in blk.instructions:
                if type(inst).__name__ == 'InstDrain' and inst.engine not in proto:
                    proto[inst.engine] = inst
    for f in nc.m.functions:
        for blk in f.blocks:
            il = blk.instructions
            i = 0
            while i < len(il):
                inst = il[i]
                si = inst.sync_info
                if si is None or len(si.on_wait) <= 1:
                    i += 1
                    continue
                waits = list(si.on_wait)
                eng = inst.engine
                for w in waits[:-1]:
                    nop = copy.deepcopy(proto[eng])
                    nop.name = f"I-ws{nc.next_id()}"
                    nop.sync_info = type(si)(on_wait=[w], on_update=[])
                    il.insert(i, nop)
                    i += 1
                inst.sync_info = type(si)(
                    on_wait=[waits[-1]], on_update=list(si.on_update)
                )
                i += 1


_NC = {}


def _get_nc(split=True):
    if split not in _NC:
        nc = bass.Bass()
        with tile.TileContext(nc) as tc:
            _emit(tc)
        if split:
            _split_waits(nc)
            _split_evsem_clears(nc)
        _NC[split] = nc
    return _NC[split]


def _host_inputs(w1, b1, w2, b2, conv_w, conv_b):
    import ml_dtypes
    w1 = np.asarray(w1, np.float32)
    w2 = np.asarray(w2, np.float32)
    w1h = np.ascontiguousarray(w1.reshape(2, 128, HID).transpose(1, 0, 2))
    w1sh = np.ascontiguousarray(w1h / float(SP))
    w2h = np.ascontiguousarray(np.asarray(w2, np.float32).reshape(HID, 2, 128))
    b1c = np.ascontiguousarray(np.asarray(b1, np.float32).reshape(HID, 1))
    b2t = np.ascontiguousarray(np.asarray(b2, np.float32).reshape(2, 128).T)
    cw = np.asarray(conv_w, np.float32).reshape(7, 7, 2)
    bands = np.zeros((H, 14, H), np.float32)
    for ic in range(2):
        for dw in range(7):
            for dh in range(7):
                d = dh - 3  # hs - ho
                v = cw[dh, dw, ic]
                if ic == 0:
                    v = v / float(C)  # savg arrives as a raw channel sum
                if d >= 0:
                    idx = np.arange(0, H - d)
                    bands[idx + d, ic * 7 + dw, idx] = v
                else:
                    idx = np.arange(-d, H)
                    bands[idx + d, ic * 7 + dw, idx] = v
    ident = np.eye(128, dtype=np.float32)
    ident16 = np.eye(128, dtype=ml_dtypes.bfloat16)
    mask2 = np.zeros((128, 2), np.float32)
    mask2[0:64, 0] = 1.0
    mask2[64:128, 1] = 1.0
    mask2t = np.ascontiguousarray(mask2.T)
    mask2b16 = mask2.astype(ml_dtypes.bfloat16)
    convb = np.full((H, 1), np.asarray(conv_b, np.float32).reshape(-1)[0], np.float32)
    return dict(w1h=w1h, w1sh=w1sh, w2h=w2h, b1c=b1c, b2t=b2t,
                bands=bands, ident=ident, ident16=ident16, mask2=mask2b16,
                mask2t=mask2t, convb=convb)


def kernel(x, w1, b1, w2, b2, conv_w, conv_b, _trace=False):
    from concourse.bass_utils import run_bass_kernel_spmd

    nc = _get_nc()
    consts = _host_inputs(w1, b1, w2, b2, conv_w, conv_b)
    xs = np.ascontiguousarray(np.asarray(x, np.float32)).reshape(8, ROWS_CORE, C)
    in_maps = [dict(consts, x=xs[i]) for i in range(N_CORES)]
    res = run_bass_kernel_spmd(nc, in_maps, core_ids=list(range(N_CORES)),
                               trace=_trace)
    out = np.stack([r["out"] for r in res.results])  # [8, 12544, 256]
    out = out.reshape(32, H, W, C)
    if _trace:
        kernel.last_results = res
    return out
